# revision 37
# baseline (speedup 1.0000x reference)
"""Trainium2 Bass kernel for nn_EnhancedTFNLayer.

Sharding: data-parallel over batch — B=8 batch elements, one per NeuronCore.
Each core runs the full pipeline for its batch element:

  emb   = embeddings + pos_table[idx]                       (indirect DMA gather)
  K^T   = exp(-(grid_g - pos_n)^2 / (2 sigma^2))            [n, g] layout
  f^T   = sum_n emb[n, d] K^T[n, g]                         (PE, accumulated in PSUM)
  4x evolution step:
      f_gd   = transpose(f^T)                               (PE transposes)
      scores = f^T.T @ f^T  (symmetric, per 128-row block)  (PE)
      E      = exp(scale*s - scale*rowmax)   + rowsum Z     (ACT, fused accum)
      attn   = E / Z                                        (GpSimd, in place)
      attn^T = transpose(attn)                              (PE transposes)
      inter^T= f_gd.T @ attn^T                              (PE)
      f^T   += dt*alpha*lap(f^T) + dt*beta*inter^T          (DVE fused passes)
  sampled = W^T.T @ field  with W[n,g] = relu(1-|u_n-g|)    (PE; exact lerp)
  LN1 -> out_proj (PE) -> LN2 -> out (sorted order; host unpermutes)

Tokens are processed sorted by position: the interpolation matrix W then
becomes block-banded (1-2 grid blocks per 128-token tile instead of 8).
Matmul operands are float32r (fp32 rounded to 11-bit mantissa, 4x PE rate
vs plain fp32); attention internals (attn, attn^T, field in [g,d]) are
bf16 since `inter` enters the field update scaled by dt*beta=0.01.
Host precomputes gather indices, sorted u values, and the sort
permutation (exact fp32 semantics of the reference).
"""

import math
from contextlib import ExitStack

import numpy as np

import concourse.bacc as bacc
import concourse.bass as bass
import concourse.tile as tile
from concourse import mybir
from concourse.bass_utils import run_bass_kernel_spmd
from concourse.masks import make_identity
from concourse.tile_rust import add_dep_helper

P = 128
N = 2048          # tokens
D = 512           # embed dim
G = 1024          # grid points
MAXLEN = 2048     # pos table rows
NT = N // P       # 16 token tiles
DB = D // P       # 4 embed blocks
GB = G // P       # 8 grid blocks
RK = 64           # Chebyshev rank of the RBF kernel factorization
STEPS = 4
LN_EPS = 1e-5
NCORES = 8

F32 = mybir.dt.float32
F32R = mybir.dt.float32r
BF16 = mybir.dt.bfloat16
FP8 = mybir.dt.float8e4
DR = mybir.MatmulPerfMode.DoubleRow
I32 = mybir.dt.int32
AX = mybir.AxisListType
ALU = mybir.AluOpType
ACTF = mybir.ActivationFunctionType


def _r(ap):
    """Bitcast fp32 AP to float32r for full-rate PE matmul."""
    return ap.bitcast(F32R)


def ts(i, size):
    return slice(i * size, (i + 1) * size)

def build_nc(c_exp, scale, beta_dt, ln1_trivial=True, ln2_trivial=True,
             bout_trivial=True, bands=None, pt_bands=None, dbg_steps=STEPS,
             dbg_do_f=True):
    """Build and compile the per-core Bass program.

    c_exp   : -1/(2 sigma^2) as float (fp32-rounded)
    scale   : 1/sqrt(D) as float
    beta_dt : beta*dt as float
    bands   : per token-tile tuple of grid blocks its sorted positions
              touch (sampling matmul band sparsity)
    pt_bands: per token-tile tuple of 128-row pos_table blocks its
              (sorted) idx values touch (one-hot gather matmul)
    """
    if bands is None:
        bands = tuple(tuple(range(GB)) for _ in range(NT))
    if pt_bands is None:
        pt_bands = tuple(tuple(range(MAXLEN // P)) for _ in range(NT))
    nc = bacc.Bacc()

    # ---- I/O ----
    # emb0 rows are pre-sorted by position on the host (plain DMA load)
    emb0_d = nc.declare_dram_parameter("emb0", [N, D], F32, isOutput=False)
    # pos_table in bf16 (entries ~0.02; bf16 error is negligible)
    pt_d = nc.declare_dram_parameter("pt", [MAXLEN, D], BF16, isOutput=False)
    # low-rank RBF factorization: K = A @ B^T (Chebyshev, exact to 1e-15)
    bm_d = nc.declare_dram_parameter("bmat", [P, NT * RK], F32R,
                                     isOutput=False)
    at_d = nc.declare_dram_parameter("amatT", [RK, G], F32R, isOutput=False)
    idxrow_d = nc.declare_dram_parameter("idxrow", [1, N], F32,
                                         isOutput=False)
    urow_d = nc.declare_dram_parameter("urow", [1, N], F32, isOutput=False)
    giota_d = nc.declare_dram_parameter("giota", [MAXLEN, 1], F32,
                                       isOutput=False)
    adt_d = nc.declare_dram_parameter("alphadt", [D, 1], F32, isOutput=False)
    wout_d = nc.declare_dram_parameter("wout", [D, D], F32R, isOutput=False)
    if not ln1_trivial:
        ln1g_d = nc.declare_dram_parameter("ln1g", [1, D], F32, isOutput=False)
        ln1b_d = nc.declare_dram_parameter("ln1b", [1, D], F32, isOutput=False)
    if not ln2_trivial:
        ln2g_d = nc.declare_dram_parameter("ln2g", [1, D], F32, isOutput=False)
        ln2b_d = nc.declare_dram_parameter("ln2b", [1, D], F32, isOutput=False)
    if not bout_trivial:
        bout_d = nc.declare_dram_parameter("bout", [1, D], F32, isOutput=False)
    out_d = nc.declare_dram_parameter("out", [N, D], F32, isOutput=True)


    with tile.TileContext(nc) as tc:
      with tc.tile_pool(name="const", bufs=1) as const, \
           tc.tile_pool(name="colp", bufs=48) as colp, \
           tc.tile_pool(name="ffinp", bufs=1) as ffinp, \
           tc.tile_pool(name="embp", bufs=16) as embp:
        # ---- constants (live for the whole kernel) ----
        ident = const.tile([P, P], F32, name="ident", tag="ident")
        make_identity(nc, ident[:])
        identr = const.tile([P, P], F32R, name="identr", tag="identr")
        nc.scalar.copy(identr[:], ident[:])
        identb = const.tile([P, P], BF16, name="identb", tag="identb")
        nc.scalar.copy(identb[:], ident[:])
        # critical-path loads first: idx row broadcast (one-hot pt gather),
        # B factor, row-block iotas
        idxb = const.tile([P, N], F32, name="idxb", tag="idxb")
        nc.sync.dma_start(idxb[:], idxrow_d[:, :].to_broadcast((P, N)))
        # B factor, all 16 token tiles batched: [128, nt*RK + j]
        ball = const.tile([P, NT * RK], F32R, name="ball", tag="ball")
        nc.sync.dma_start(ball[:], bm_d[:, :])
        giota_all = const.tile([P, MAXLEN // P], F32, name="giota_all",
                               tag="giota_all")
        nc.sync.dma_start(giota_all[:],
                          giota_d[:, :].rearrange("(a b) c -> b (a c)", b=P))
        adt_col = []
        for db in range(DB):
            a = const.tile([P, 1], F32, name=f"adt{db}", tag=f"adt{db}")
            nc.sync.dma_start(a[:], adt_d[ts(db, P), :])
            adt_col.append(a)
        c0_col = []
        for db in range(DB):
            c = const.tile([P, 1], F32, name=f"c0_{db}", tag=f"c0_{db}")
            nc.vector.tensor_scalar(out=c[:], in0=adt_col[db][:],
                                    scalar1=-2.0, scalar2=1.0,
                                    op0=ALU.mult, op1=ALU.add)
            c0_col.append(c)
        eps_col = const.tile([P, 1], F32, name="eps", tag="eps")
        nc.vector.memset(eps_col[:], LN_EPS)
        if not ln1_trivial:
            g1row = const.tile([P, D], F32, name="g1row", tag="g1row")
            nc.sync.dma_start(g1row[:], ln1g_d[:, :].to_broadcast((P, D)))
            b1row = const.tile([P, D], F32, name="b1row", tag="b1row")
            nc.sync.dma_start(b1row[:], ln1b_d[:, :].to_broadcast((P, D)))
        if not ln2_trivial:
            g2row = const.tile([P, D], F32, name="g2row", tag="g2row")
            nc.sync.dma_start(g2row[:], ln2g_d[:, :].to_broadcast((P, D)))
            b2row = const.tile([P, D], F32, name="b2row", tag="b2row")
            nc.sync.dma_start(b2row[:], ln2b_d[:, :].to_broadcast((P, D)))
        if not bout_trivial:
            boutrow = const.tile([P, D], F32, name="boutrow", tag="boutrow")
            nc.sync.dma_start(boutrow[:], bout_d[:, :].to_broadcast((P, D)))

        emb_sb = []  # resident f32r emb tiles (sorted order)
        ffin = []   # final field [g, d] tiles, f32r, kept for sampling

        # ======== evolution scope: pools freed before phase F ========
        with tc.tile_pool(name="wrk", bufs=3) as wrk, \
             tc.tile_pool(name="smp", bufs=3) as smp, \
             tc.tile_pool(name="ftp", bufs=2) as ftp, \
             tc.tile_pool(name="fgdp", bufs=9) as fgdp, \
             tc.tile_pool(name="epool", bufs=9) as epool, \
             tc.tile_pool(name="atp", bufs=9) as atp, \
             tc.tile_pool(name="updp", bufs=3) as updp:

            # ---- phase B/C: emb build + low-rank field^T projection ----
            # pos_table rows fetched by a banded one-hot matmul against a
            # linearly-DMA'd bf16 table (no SWDGE); then
            # field^T = (A @ (B^T emb))^T: M = B^T emb [RK, D], fT = M^T A^T
            with tc.tile_pool(name="psA", bufs=1, space="PSUM") as psA, \
                 tc.tile_pool(name="psM", bufs=2, space="PSUM") as psMp, \
                 tc.tile_pool(name="ptp", bufs=1) as ptp, \
                 tc.tile_pool(name="ohp", bufs=4) as ohp:
                accM = psA.tile([RK, D], F32, name="accM", tag="accM")
                # pos_table tiles: DMA issued lazily in consumption order
                pt_sb = [None] * (MAXLEN // P)

                def pt_tile(k):
                    if pt_sb[k] is None:
                        t = ptp.tile([P, D], BF16, name=f"pt{k}",
                                     tag=f"pt{k}")
                        nc.sync.dma_start(t[:], pt_d[ts(k, P), :])
                        pt_sb[k] = t
                    return pt_sb[k]

                for nt in range(NT):
                    # one-hot gather: ptg = onehot(idx)^T @ pos_table
                    ps_pt = psMp.tile([P, D], F32, name="ps_pt", tag="ps_pt")
                    bl = pt_bands[nt]
                    for bi, k in enumerate(bl):
                        oh = ohp.tile([P, P], BF16, name="oh", tag="oh")
                        nc.vector.tensor_scalar(
                            out=oh[:], in0=idxb[:, ts(nt, P)],
                            scalar1=giota_all[:, k:k + 1], scalar2=None,
                            op0=ALU.is_equal)
                        nc.tensor.matmul(ps_pt[:], lhsT=oh[:],
                                         rhs=pt_tile(k)[:],
                                         start=(bi == 0),
                                         stop=(bi == len(bl) - 1))
                    # emb = emb0_sorted + pos_table[idx]  (sorted order);
                    # the DVE add writes f32r (rounding producer)
                    embt = smp.tile([P, D], F32, name="embt", tag="embt")
                    nc.sync.dma_start(embt[:], emb0_d[ts(nt, P), :])
                    embr = embp.tile([P, D], F32R, name="embr", tag="embr")
                    nc.vector.tensor_add(embr[:], embt[:], ps_pt[:])
                    emb_sb.append(embr)
                    # accumulate M[r, d] over token tiles
                    nc.tensor.matmul(
                        accM[:],
                        lhsT=ball[:, nt * RK:(nt + 1) * RK],
                        rhs=embr[:],
                        start=(nt == 0), stop=(nt == NT - 1))
                msb = wrk.tile([RK, D], F32R, name="msb", tag="msb")
                nc.scalar.copy(msb[:], accM[:])

                at_sb = const.tile([RK, G], F32R, name="at_sb", tag="at_sb")
                nc.sync.dma_start(at_sb[:], at_d[:, :])
                # field^T tiles [128 d, 1024 g], stored f32r (PE-ready);
                # elementwise readers use .bitcast(F32) views
                atr = at_sb[:]
                fT = []
                for db in range(DB):
                    t = ftp.tile([P, G], F32R, name=f"fT{db}", tag=f"fT{db}")
                    for gc in range(2):
                        pm = psMp.tile([P, 512], F32, name="pm", tag="pm")
                        nc.tensor.matmul(pm[:], lhsT=msb[:, ts(db, P)],
                                         rhs=atr[:, ts(gc, 512)],
                                         start=True, stop=True)
                        nc.scalar.copy(t[:, ts(gc, 512)], pm[:])
                    fT.append(t)

            # ---- phase D: evolution ----
            # one shared PSUM ring: [128, 1024] f32 tiles (2 banks x 4 bufs
            # = all 8 banks); other layouts use bitcast views of it
            with tc.tile_pool(name="psB", bufs=4, space="PSUM") as psB, \
                 tc.tile_pool(name="dzp", bufs=10) as dzp:

                for step in range(dbg_steps):
                    # 1b. lap term t3 = f + dt*alpha*lap(f)
                    #     = (1-2*adt)*f + adt*f_up + adt*f_dn, three fused
                    #     passes; depends only on fT, so it overlaps scores
                    t3s = []
                    for db in range(DB):
                        f = fT[db][:].bitcast(F32)
                        adt = adt_col[db][:]
                        z = updp.tile([P, G], F32, name="upd", tag="upd",
                                      bufs=5)
                        nc.gpsimd.tensor_scalar_mul(z[:], f, c0_col[db][:])
                        # += adt*f_up  (edge: f_up[G-1] = f[G-1])
                        nc.vector.scalar_tensor_tensor(
                            out=z[:, 0:G - 1], in0=f[:, 1:G], scalar=adt,
                            in1=z[:, 0:G - 1], op0=ALU.mult, op1=ALU.add)
                        nc.vector.scalar_tensor_tensor(
                            out=z[:, G - 1:G], in0=f[:, G - 1:G], scalar=adt,
                            in1=z[:, G - 1:G], op0=ALU.mult, op1=ALU.add)
                        # += adt*f_dn  (edge: f_dn[0] = f[0])
                        nc.vector.scalar_tensor_tensor(
                            out=z[:, 1:G], in0=f[:, 0:G - 1], scalar=adt,
                            in1=z[:, 1:G], op0=ALU.mult, op1=ALU.add)
                        nc.vector.scalar_tensor_tensor(
                            out=z[:, 0:1], in0=f[:, 0:1], scalar=adt,
                            in1=z[:, 0:1], op0=ALU.mult, op1=ALU.add)
                        t3s.append(z)

                    # 2. scores + softmax per grid block (attn in bf16)
                    attn = []
                    for gb in range(GB):
                        ps_s = psB.tile([P, G], F32, name="big", tag="big")
                        for db in range(DB):
                            for hc in range(2):
                                nc.tensor.matmul(
                                    ps_s[:, ts(hc, 512)],
                                    lhsT=fT[db][:, ts(gb, P)],
                                    rhs=fT[db][:, ts(hc, 512)],
                                    start=(db == 0), stop=(db == DB - 1))
                        # stabilizer: rowmax over every 4th column; S rows
                        # are smooth in h (field correlation ~200 grid pts),
                        # so the bias is at most ~1 exp-unit below the true
                        # max -- harmless (E <= e)
                        m_col = colp.tile([P, 1], F32, name="col", tag="col")
                        nc.vector.tensor_reduce(m_col[:], ps_s[:, 0:G:4],
                                                axis=AX.X, op=ALU.max)
                        negm = colp.tile([P, 1], F32, name="col", tag="col")
                        nc.vector.tensor_scalar_mul(negm[:], m_col[:],
                                                    -scale)
                        zcol = colp.tile([P, 1], F32, name="col", tag="col")
                        et = epool.tile([P, G], BF16, name="E", tag="E")
                        nc.scalar.activation(et[:], ps_s[:], ACTF.Exp,
                                             bias=negm[:], scale=scale,
                                             accum_out=zcol[:])
                        zinv = colp.tile([P, 1], F32, name="col", tag="col")
                        nc.vector.reciprocal(zinv[:], zcol[:])
                        # normalization is folded into the transpose as a
                        # diag(zinv) matmul: build the diagonal now
                        dz = dzp.tile([P, P], BF16, name="dz", tag="dz")
                        nc.vector.tensor_scalar_mul(dz[:], identb[:],
                                                    zinv[:])
                        attn.append((et, dz))

                    # 1. field in [g, d] layout, fp8 hb-PAIR tiles
                    #    [128, 2, 512] (DoubleRow lhsT for inter; inter
                    #    enters the update scaled by dt*beta)
                    fgd = []
                    for pr in range(GB // 2):
                        pst = psB.tile([P, G], F32, name="big", tag="big")
                        pr_r = pst[:].bitcast(F32R)
                        for j in range(2):
                            for db in range(DB):
                                nc.tensor.transpose(
                                    pr_r[:, j * 512 + db * P:
                                         j * 512 + (db + 1) * P],
                                    fT[db][:, ts(2 * pr + j, P)],
                                    identr[:])
                        t = fgdp.tile([P, 2, 512], FP8, name="fgd",
                                      tag="fgd")
                        nc.scalar.copy(
                            t[:], pr_r.rearrange("p (b c) -> p b c", b=2))
                        fgd.append(t)

                    # 3. attn^T with the softmax normalization fused in:
                    #    attnT[h, g] = E[g, h] * zinv[g] = (E_chunk.T @
                    #    diag(zinv)) -- a regular bf16 matmul per block,
                    #    same PE cost as a transpose; fp32 psum -> fp8 copy
                    attnT = []
                    for pr in range(GB // 2):
                        at = atp.tile([P, 2, G], FP8, name="aT", tag="aT")
                        attnT.append(at)
                    for hb in range(GB):
                        pst = psB.tile([P, G], F32, name="big", tag="big")
                        for gb in range(GB):
                            et, dz = attn[gb]
                            nc.tensor.matmul(
                                pst[:, ts(gb, P)],
                                lhsT=et[:, ts(hb, P)], rhs=dz[:],
                                start=True, stop=True)
                        if hb % 2 == 0:
                            nc.scalar.copy(attnT[hb // 2][:, hb % 2, :],
                                           pst[:])
                        else:
                            nc.vector.tensor_copy(
                                attnT[hb // 2][:, hb % 2, :], pst[:])


                    # 4. inter^T (fp8 DoubleRow: 256 contraction rows/mm)
                    #    + field update per embed block
                    fT_new = []
                    for db in range(DB):
                        ps_i = psB.tile([P, G], F32, name="big", tag="big")
                        for pr in range(GB // 2):
                            for gc in range(2):
                                nc.tensor.matmul(
                                    ps_i[:, ts(gc, 512)],
                                    lhsT=fgd[pr][:, :, ts(db, P)],
                                    rhs=attnT[pr][:, :, ts(gc, 512)],
                                    start=(pr == 0), stop=(pr == GB // 2 - 1),
                                    perf_mode=DR)
                        # f_new = beta*dt*inter + t3  (written f32r-rounded)
                        fn = ftp.tile([P, G], F32R, name=f"fT{db}",
                                      tag=f"fT{db}")
                        nc.vector.scalar_tensor_tensor(
                            out=fn[:], in0=ps_i[:], scalar=beta_dt,
                            in1=t3s[db][:], op0=ALU.mult, op1=ALU.add)
                        fT_new.append(fn)
                    fT = fT_new

                # ---- phase E: final field -> [g, d] f32r tiles in SBUF ----
                for pr in range(GB // 2):
                    pst = psB.tile([P, G], F32, name="big", tag="big")
                    pr_r = pst[:].bitcast(F32R)
                    for j in range(2):
                        gb = 2 * pr + j
                        for db in range(DB):
                            nc.tensor.transpose(
                                pr_r[:, j * 512 + db * P:
                                     j * 512 + (db + 1) * P],
                                fT[db][:, ts(gb, P)], identr[:])
                    t = ffinp.tile([P, 2, 512], F32R, name=f"ffin{pr}",
                                   tag=f"ffin{pr}")
                    nc.scalar.copy(
                        t[:], pr_r.rearrange("p (b c) -> p b c", b=2))
                    ffin.append(t)

        # ======== phase F scope (evolution SBUF freed) ========
        inv_d = 1.0 / D
        with tc.tile_pool(name="wtp", bufs=1) as wtp, \
             tc.tile_pool(name="lnp", bufs=4) as lnp, \
             tc.tile_pool(name="psF", bufs=2, space="PSUM") as psF, \
             tc.tile_pool(name="psG", bufs=2, space="PSUM") as psG:
            # output-projection weights, f32r via bitcast (no copy needed)
            w_sb = []
            for db in range(DB):
                w = wtp.tile([P, D], F32R, name=f"wst{db}", tag=f"wst{db}")
                nc.sync.dma_start(w[:], wout_d[ts(db, P), :])
                w_sb.append(w[:])
            # NEGATED interpolation matrix -W^T[g, n] = min(|u_n-g|,1)-1 =
            # -relu(1 - |u_n - g|): exactly minus the (1-w, w) linear-interp
            # weights (stage_a compensates with a negated residual op).
            # Only the banded [128, 128] blocks are nonzero (sorted tokens)
            u_bcast = wtp.tile([P, N], F32, name="u_bcast", tag="u_bcast")
            nc.sync.dma_start(u_bcast[:], urow_d[:, :].to_broadcast((P, N)))
            wblk = {}
            for nt in range(NT if dbg_do_f else 0):
                for gb in bands[nt]:
                    q = wtp.tile([P, P], F32, name="wq", tag="wq", bufs=3)
                    nc.vector.tensor_scalar_sub(q[:],
                                                u_bcast[:, ts(nt, P)],
                                                giota_all[:, gb:gb + 1])
                    nc.scalar.activation(q[:], q[:], ACTF.Abs)
                    wt = wtp.tile([P, P], F32R, name=f"wb{nt}_{gb}",
                                  tag=f"wb{nt}_{gb}")
                    nc.vector.tensor_scalar(
                        out=wt[:], in0=q[:],
                        scalar1=1.0, scalar2=1.0,
                        op0=ALU.min, op1=ALU.subtract)
                    wblk[(nt, gb)] = wt

            def ln_stats(src, ssum, ssq_engine):
                """mean/rstd/bias from row sum + sum of squares.
                Returns (rstd, nb) col APs: norm = src*rstd + nb."""
                ssq = colp.tile([P, 1], F32, name="col", tag="col")
                scr = lnp.tile([P, D], F32, name="scr", tag="scr")
                if ssq_engine == "act":
                    nc.scalar.activation(scr[:], src[:], ACTF.Square,
                                         accum_out=ssq[:])
                else:
                    nc.vector.scalar_tensor_tensor(
                        out=scr[:], in0=src[:], scalar=1.0,
                        in1=src[:], op0=ALU.mult, op1=ALU.mult,
                        accum_out=ssq[:])
                nmean = colp.tile([P, 1], F32, name="col", tag="col")
                nc.gpsimd.tensor_scalar_mul(nmean[:], ssum[:], -inv_d)
                msq = colp.tile([P, 1], F32, name="col", tag="col")
                nc.gpsimd.tensor_mul(msq[:], nmean[:], nmean[:])
                v = colp.tile([P, 1], F32, name="col", tag="col")
                nc.vector.scalar_tensor_tensor(
                    out=v[:], in0=ssq[:], scalar=inv_d, in1=msq[:],
                    op0=ALU.mult, op1=ALU.subtract)
                sstd = colp.tile([P, 1], F32, name="col", tag="col")
                nc.scalar.activation(sstd[:], v[:], ACTF.Sqrt,
                                     bias=eps_col[:])
                rstd = colp.tile([P, 1], F32, name="col", tag="col")
                nc.vector.reciprocal(rstd[:], sstd[:])
                nb = colp.tile([P, 1], F32, name="col", tag="col")
                nc.gpsimd.tensor_mul(nb[:], nmean[:], rstd[:])
                return rstd, nb

            def stage_a(nt):
                """sample + residual -> enh (f32r).

                With trivial LN1, LN1's rstd cancels through the (host-
                folded) W+I projection because LN2 is invariant to a
                per-row positive scale: enh = xx - mean(xx) suffices."""
                ema = emb_sb[nt][:].bitcast(F32)
                # sampled = W^T.T @ field   [128 tok, 512 d]
                # (band-sparse: sorted tokens touch only bands[nt] blocks)
                ps_sm = psF.tile([P, 512], F32, name="smp", tag="smp",
                                 bufs=4)
                bl = bands[nt]
                for bi, gb in enumerate(bl):
                    nc.tensor.matmul(ps_sm[:],
                                     lhsT=wblk[(nt, gb)][:],
                                     rhs=ffin[gb // 2][:, gb % 2, :],
                                     start=(bi == 0),
                                     stop=(bi == len(bl) - 1))
                # x = sampled + emb ; accum row-sum for LN1 mean
                # (wblk holds -W, so psum is -sampled: negate here)
                xx = lnp.tile([P, D], F32, name="xx", tag="xx", bufs=8)
                ssum = colp.tile([P, 1], F32, name="col", tag="col")
                nc.vector.scalar_tensor_tensor(
                    out=xx[:], in0=ps_sm[:], scalar=-1.0,
                    in1=ema, op0=ALU.mult, op1=ALU.add,
                    accum_out=ssum[:])
                enh = lnp.tile([P, D], F32R, name="enh", tag="enh", bufs=3)
                if ln1_trivial:
                    nmean = colp.tile([P, 1], F32, name="col", tag="col")
                    nc.gpsimd.tensor_scalar_mul(nmean[:], ssum[:], -inv_d)
                    nc.scalar.activation(enh[:], xx[:], ACTF.Identity,
                                         bias=nmean[:])
                else:
                    rstd, nb = ln_stats(xx, ssum, "act")
                    nc.scalar.activation(enh[:], xx[:], ACTF.Identity,
                                         scale=rstd[:], bias=nb[:])
                    enhf = enh[:].bitcast(F32)
                    nc.gpsimd.tensor_mul(enhf, enhf, g1row[:])
                    nc.gpsimd.tensor_add(enhf, enhf, b1row[:])
                return enh

            def stage_b(nt, enh):
                """out_proj (W+I folded on host: residual included) + LN2
                -> DRAM."""
                ps_e = psG.tile([P, 512], F32R, name="sm", tag="sm")
                for db in range(DB):
                    nc.tensor.transpose(ps_e[:, ts(db, P)],
                                        enh[:, ts(db, P)], identr[:])
                enhT = lnp.tile([P, D], F32R, name="enhT", tag="enhT")
                nc.vector.tensor_copy(enhT[:], ps_e[:])
                ps_o = psF.tile([P, 512], F32, name="big", tag="big",
                                bufs=2)
                for db in range(DB):
                    nc.tensor.matmul(ps_o[:],
                                     lhsT=enhT[:, ts(db, P)],
                                     rhs=w_sb[db][:],
                                     start=(db == 0), stop=(db == DB - 1))
                # y already includes the residual via W+I (+ b_out) ; LN2
                yy = lnp.tile([P, D], F32, name="yy", tag="xx", bufs=8)
                ysum = colp.tile([P, 1], F32, name="col", tag="col")
                if bout_trivial:
                    nc.vector.tensor_scalar(
                        out=yy[:], in0=ps_o[:], scalar1=1.0, scalar2=0.0,
                        op0=ALU.mult, op1=ALU.add, accum_out=ysum[:])
                else:
                    nc.gpsimd.tensor_add(yy[:], ps_o[:], boutrow[:])
                    nc.vector.tensor_reduce(ysum[:], yy[:], axis=AX.X,
                                            op=ALU.add)
                rstd2, nb2 = ln_stats(yy, ysum, "dve")
                res = lnp.tile([P, D], F32, name="res", tag="res")
                nc.scalar.activation(res[:], yy[:], ACTF.Identity,
                                     scale=rstd2[:], bias=nb2[:])
                if not ln2_trivial:
                    nc.gpsimd.tensor_mul(res[:], res[:], g2row[:])
                    nc.gpsimd.tensor_add(res[:], res[:], b2row[:])
                # rows stay in sorted-token order; host unpermutes
                nc.sync.dma_start(out_d[ts(nt, P), :], res[:])

            # software-pipelined with skew 2: PE runs sampled-matmuls of
            # nt+1/nt+2 while the LN chain of nt completes
            SKEW = 3
            nF = NT if dbg_do_f else 0
            pend = []
            for nt in range(nF):
                pend.append((nt, stage_a(nt)))
                if len(pend) > SKEW:
                    j, e = pend.pop(0)
                    stage_b(j, e)
            for j, e in pend:
                stage_b(j, e)

    nc.compile()
    return nc


def host_prep(embeddings, positions, grid_points, pos_table, sigma, alpha,
              beta, dt, ln1_g, ln1_b, ln2_g, ln2_b, w_out, b_out):
    """Host-side prep: derived index tensors + per-core input maps."""
    embeddings = np.asarray(embeddings, np.float32)
    positions = np.asarray(positions, np.float32)
    grid_points = np.asarray(grid_points, np.float32)
    pos_table = np.ascontiguousarray(np.asarray(pos_table, np.float32))
    alpha = np.asarray(alpha, np.float32)
    # residual fold: out+enh = enh @ (W + I)
    w_out = np.ascontiguousarray(np.asarray(w_out, np.float32)
                                 + np.eye(D, dtype=np.float32))
    b_out = np.asarray(b_out, np.float32)
    sigma = np.float32(np.asarray(sigma))
    beta = np.float32(np.asarray(beta))
    dt = np.float32(np.asarray(dt))
    ln1_g = np.asarray(ln1_g, np.float32)
    ln1_b = np.asarray(ln1_b, np.float32)
    ln2_g = np.asarray(ln2_g, np.float32)
    ln2_b = np.asarray(ln2_b, np.float32)

    c_exp = float(-(np.float32(1.0) / (np.float32(2.0) * sigma * sigma)))
    scale = float(np.float32(1.0) / np.sqrt(np.float32(D)))
    beta_dt = float(beta * dt)
    alphadt = np.ascontiguousarray((dt * alpha).astype(np.float32)
                                   .reshape(D, 1))

    # Chebyshev factorization of the RBF kernel: K(u,p) = L(u) Kc L(p)^T,
    # exact to ~1e-15 at RK nodes (kernel is entire, sigma=0.2 wide)
    kq = np.arange(RK)
    tn = 0.5 + 0.5 * np.cos((2 * kq + 1) * np.pi / (2 * RK))
    bw = np.empty(RK)
    for j in range(RK):
        bw[j] = 1.0 / np.prod(tn[j] - np.delete(tn, j))

    def lagrange(x):
        diff = x[:, None] - tn[None, :]
        hit = np.isclose(diff, 0.0, atol=1e-14)
        diff = np.where(hit, 1.0, diff)
        num = bw[None, :] / diff
        L = num / num.sum(1, keepdims=True)
        rows = hit.any(1)
        L[rows] = hit[rows].astype(np.float64)
        return L

    ln1_trivial = bool(np.all(ln1_g == 1.0) and np.all(ln1_b == 0.0))
    ln2_trivial = bool(np.all(ln2_g == 1.0) and np.all(ln2_b == 0.0))
    bout_trivial = bool(np.all(b_out == 0.0))

    import ml_dtypes

    def to_f32r(x):
        xb = np.asarray(x, np.float32).view(np.uint32)
        sh = np.uint32(13)
        r = ((xb >> sh) + ((xb >> np.uint32(12)) & np.uint32(1))) << sh
        return r.view(np.float32)

    pt_bf16 = np.ascontiguousarray(pos_table.astype(ml_dtypes.bfloat16))
    giota = np.arange(MAXLEN, dtype=np.float32).reshape(MAXLEN, 1)
    in_maps = []
    all_bands = []
    all_pt_bands = []
    all_orders = []
    for c in range(NCORES):
        pos_n = positions[c, :, 0]                     # [N] fp32 (natural)
        u_n = pos_n * np.float32(G - 1)
        order = np.argsort(u_n, kind="stable").astype(np.int32)
        all_orders.append(order)
        pos = pos_n[order]                             # sorted token order
        u = u_n[order]
        idx = np.clip(np.rint(pos * np.float32(MAXLEN - 1)).astype(np.int32),
                      0, MAXLEN - 1)
        # grid blocks each sorted token tile touches (i0..i0+1 support)
        i0 = np.clip(np.floor(u).astype(np.int64), 0, G - 1)
        ihi = np.minimum(i0 + 1, G - 1)
        bands = []
        for nt in range(NT):
            lo = int(i0[nt * P:(nt + 1) * P].min()) // P
            hi = int(ihi[nt * P:(nt + 1) * P].max()) // P
            bands.append(tuple(range(lo, hi + 1)))
        all_bands.append(tuple(bands))
        ptb = []
        for nt in range(NT):
            lo = int(idx[nt * P:(nt + 1) * P].min()) // P
            hi = int(idx[nt * P:(nt + 1) * P].max()) // P
            ptb.append(tuple(range(lo, hi + 1)))
        all_pt_bands.append(tuple(ptb))
        u_g = grid_points[c, :, 0].astype(np.float64)
        amatT = (np.exp(-(tn[:, None] - tn[None, :]) ** 2
                        / (2.0 * float(sigma) ** 2)) @ lagrange(u_g).T)
        bmat = lagrange(pos.astype(np.float64))
        m = {
            "emb0": np.ascontiguousarray(embeddings[c][order]),
            "pt": pt_bf16,
            "bmat": np.ascontiguousarray(to_f32r(
                bmat.astype(np.float32).reshape(NT, P, RK)
                .transpose(1, 0, 2).reshape(P, NT * RK))),
            "amatT": np.ascontiguousarray(to_f32r(amatT)),
            "idxrow": np.ascontiguousarray(
                idx.astype(np.float32).reshape(1, N)),
            "urow": np.ascontiguousarray(u.reshape(1, N)),
            "giota": giota,
            "alphadt": alphadt,
            "wout": to_f32r(w_out),
        }
        if not ln1_trivial:
            m["ln1g"] = np.ascontiguousarray(ln1_g.reshape(1, D))
            m["ln1b"] = np.ascontiguousarray(ln1_b.reshape(1, D))
        if not ln2_trivial:
            m["ln2g"] = np.ascontiguousarray(ln2_g.reshape(1, D))
            m["ln2b"] = np.ascontiguousarray(ln2_b.reshape(1, D))
        if not bout_trivial:
            m["bout"] = np.ascontiguousarray(b_out.reshape(1, D))
        in_maps.append(m)

    # SPMD: one program for all cores -> per-tile band = union over cores
    bands = tuple(
        tuple(range(min(b[nt][0] for b in all_bands),
                    max(b[nt][-1] for b in all_bands) + 1))
        for nt in range(NT))
    pt_bands = tuple(
        tuple(range(min(b[nt][0] for b in all_pt_bands),
                    max(b[nt][-1] for b in all_pt_bands) + 1))
        for nt in range(NT))
    build_key = (c_exp, scale, beta_dt, ln1_trivial, ln2_trivial,
                 bout_trivial, bands, pt_bands)
    return in_maps, build_key, all_orders


_NC_CACHE = {}


def kernel(**inputs):
    in_maps, build_key, orders = host_prep(**inputs)
    if build_key not in _NC_CACHE:
        _NC_CACHE[build_key] = build_nc(*build_key)
    nc = _NC_CACHE[build_key]
    res = run_bass_kernel_spmd(nc, in_maps, list(range(NCORES)))
    out = np.empty((NCORES, N, D), np.float32)
    for i in range(NCORES):
        out[i, orders[i], :] = res.results[i]["out"]
    return out



# revision 38
# speedup vs baseline: 1.0585x; 1.0585x over previous
"""Trainium2 Bass kernel for nn_EnhancedTFNLayer.

Sharding: data-parallel over batch — B=8 batch elements, one per NeuronCore.
Each core runs the full pipeline for its batch element:

  emb   = embeddings + pos_table[idx]                       (indirect DMA gather)
  K^T   = exp(-(grid_g - pos_n)^2 / (2 sigma^2))            [n, g] layout
  f^T   = sum_n emb[n, d] K^T[n, g]                         (PE, accumulated in PSUM)
  4x evolution step:
      f_gd   = transpose(f^T)                               (PE transposes)
      scores = f^T.T @ f^T  (symmetric, per 128-row block)  (PE)
      E      = exp(scale*s - scale*rowmax)   + rowsum Z     (ACT, fused accum)
      attn   = E / Z                                        (GpSimd, in place)
      attn^T = transpose(attn)                              (PE transposes)
      inter^T= f_gd.T @ attn^T                              (PE)
      f^T   += dt*alpha*lap(f^T) + dt*beta*inter^T          (DVE fused passes)
  sampled = W^T.T @ field  with W[n,g] = relu(1-|u_n-g|)    (PE; exact lerp)
  LN1 -> out_proj (PE) -> LN2 -> out (sorted order; host unpermutes)

Tokens are processed sorted by position: the interpolation matrix W then
becomes block-banded (1-2 grid blocks per 128-token tile instead of 8).
Matmul operands are float32r (fp32 rounded to 11-bit mantissa, 4x PE rate
vs plain fp32); attention internals (attn, attn^T, field in [g,d]) are
bf16 since `inter` enters the field update scaled by dt*beta=0.01.
Host precomputes gather indices, sorted u values, and the sort
permutation (exact fp32 semantics of the reference).
"""

import math
from contextlib import ExitStack

import numpy as np

import concourse.bacc as bacc
import concourse.bass as bass
import concourse.tile as tile
from concourse import mybir
from concourse.bass_utils import run_bass_kernel_spmd
from concourse.masks import make_identity
from concourse.tile_rust import add_dep_helper

P = 128
N = 2048          # tokens
D = 512           # embed dim
G = 1024          # grid points
MAXLEN = 2048     # pos table rows
NT = N // P       # 16 token tiles
DB = D // P       # 4 embed blocks
GB = G // P       # 8 grid blocks
RK = 64           # Chebyshev rank of the RBF kernel factorization
STEPS = 4
LN_EPS = 1e-5
NCORES = 8

F32 = mybir.dt.float32
F32R = mybir.dt.float32r
BF16 = mybir.dt.bfloat16
FP8 = mybir.dt.float8e4
DR = mybir.MatmulPerfMode.DoubleRow
I32 = mybir.dt.int32
AX = mybir.AxisListType
ALU = mybir.AluOpType
ACTF = mybir.ActivationFunctionType


def _r(ap):
    """Bitcast fp32 AP to float32r for full-rate PE matmul."""
    return ap.bitcast(F32R)


def ts(i, size):
    return slice(i * size, (i + 1) * size)

def build_nc(c_exp, scale, beta_dt, ln1_trivial=True, ln2_trivial=True,
             bout_trivial=True, bands=None, pt_bands=None, dbg_steps=STEPS,
             dbg_do_f=True):
    """Build and compile the per-core Bass program.

    c_exp   : -1/(2 sigma^2) as float (fp32-rounded)
    scale   : 1/sqrt(D) as float
    beta_dt : beta*dt as float
    bands   : per token-tile tuple of grid blocks its sorted positions
              touch (sampling matmul band sparsity)
    pt_bands: per token-tile tuple of 128-row pos_table blocks its
              (sorted) idx values touch (one-hot gather matmul)
    """
    if bands is None:
        bands = tuple(tuple(range(GB)) for _ in range(NT))
    if pt_bands is None:
        pt_bands = tuple(tuple(range(MAXLEN // P)) for _ in range(NT))
    nc = bacc.Bacc()

    # ---- I/O ----
    # emb0 rows are pre-sorted by position on the host (plain DMA load)
    emb0_d = nc.declare_dram_parameter("emb0", [N, D], F32, isOutput=False)
    # pos_table in bf16 (entries ~0.02; bf16 error is negligible)
    pt_d = nc.declare_dram_parameter("pt", [MAXLEN, D], BF16, isOutput=False)
    # low-rank RBF factorization: K = A @ B^T (Chebyshev, exact to 1e-15)
    bm_d = nc.declare_dram_parameter("bmat", [P, NT * RK], F32R,
                                     isOutput=False)
    at_d = nc.declare_dram_parameter("amatT", [RK, G], F32R, isOutput=False)
    idxrow_d = nc.declare_dram_parameter("idxrow", [1, N], F32,
                                         isOutput=False)
    urow_d = nc.declare_dram_parameter("urow", [1, N], F32, isOutput=False)
    giota_d = nc.declare_dram_parameter("giota", [MAXLEN, 1], F32,
                                       isOutput=False)
    adt_d = nc.declare_dram_parameter("alphadt", [D, 1], F32, isOutput=False)
    wout_d = nc.declare_dram_parameter("wout", [D, D], F32R, isOutput=False)
    if not ln1_trivial:
        ln1g_d = nc.declare_dram_parameter("ln1g", [1, D], F32, isOutput=False)
        ln1b_d = nc.declare_dram_parameter("ln1b", [1, D], F32, isOutput=False)
    if not ln2_trivial:
        ln2g_d = nc.declare_dram_parameter("ln2g", [1, D], F32, isOutput=False)
        ln2b_d = nc.declare_dram_parameter("ln2b", [1, D], F32, isOutput=False)
    if not bout_trivial:
        bout_d = nc.declare_dram_parameter("bout", [1, D], F32, isOutput=False)
    out_d = nc.declare_dram_parameter("out", [N, D], F32, isOutput=True)


    with tile.TileContext(nc) as tc:
      with tc.tile_pool(name="const", bufs=1) as const, \
           tc.tile_pool(name="colp", bufs=48) as colp, \
           tc.tile_pool(name="ffinp", bufs=1) as ffinp, \
           tc.tile_pool(name="embp", bufs=16) as embp:
        # ---- constants (live for the whole kernel) ----
        ident = const.tile([P, P], F32, name="ident", tag="ident")
        make_identity(nc, ident[:])
        identr = const.tile([P, P], F32R, name="identr", tag="identr")
        nc.scalar.copy(identr[:], ident[:])
        identb = const.tile([P, P], BF16, name="identb", tag="identb")
        nc.scalar.copy(identb[:], ident[:])
        # critical-path loads first: idx row broadcast (one-hot pt gather),
        # B factor, row-block iotas
        idxb = const.tile([P, N], F32, name="idxb", tag="idxb")
        nc.sync.dma_start(idxb[:], idxrow_d[:, :].to_broadcast((P, N)))
        # B factor, all 16 token tiles batched: [128, nt*RK + j]
        ball = const.tile([P, NT * RK], F32R, name="ball", tag="ball")
        nc.sync.dma_start(ball[:], bm_d[:, :])
        giota_all = const.tile([P, MAXLEN // P], F32, name="giota_all",
                               tag="giota_all")
        nc.sync.dma_start(giota_all[:],
                          giota_d[:, :].rearrange("(a b) c -> b (a c)", b=P))
        adt_col = []
        for db in range(DB):
            a = const.tile([P, 1], F32, name=f"adt{db}", tag=f"adt{db}")
            nc.sync.dma_start(a[:], adt_d[ts(db, P), :])
            adt_col.append(a)
        c0_col = []
        for db in range(DB):
            c = const.tile([P, 1], F32, name=f"c0_{db}", tag=f"c0_{db}")
            nc.vector.tensor_scalar(out=c[:], in0=adt_col[db][:],
                                    scalar1=-2.0, scalar2=1.0,
                                    op0=ALU.mult, op1=ALU.add)
            c0_col.append(c)
        eps_col = const.tile([P, 1], F32, name="eps", tag="eps")
        nc.vector.memset(eps_col[:], LN_EPS)
        if not ln1_trivial:
            g1row = const.tile([P, D], F32, name="g1row", tag="g1row")
            nc.sync.dma_start(g1row[:], ln1g_d[:, :].to_broadcast((P, D)))
            b1row = const.tile([P, D], F32, name="b1row", tag="b1row")
            nc.sync.dma_start(b1row[:], ln1b_d[:, :].to_broadcast((P, D)))
        if not ln2_trivial:
            g2row = const.tile([P, D], F32, name="g2row", tag="g2row")
            nc.sync.dma_start(g2row[:], ln2g_d[:, :].to_broadcast((P, D)))
            b2row = const.tile([P, D], F32, name="b2row", tag="b2row")
            nc.sync.dma_start(b2row[:], ln2b_d[:, :].to_broadcast((P, D)))
        if not bout_trivial:
            boutrow = const.tile([P, D], F32, name="boutrow", tag="boutrow")
            nc.sync.dma_start(boutrow[:], bout_d[:, :].to_broadcast((P, D)))

        emb_sb = []  # resident f32r emb tiles (sorted order)
        ffin = []   # final field [g, d] tiles, f32r, kept for sampling

        # ======== evolution scope: pools freed before phase F ========
        with tc.tile_pool(name="wrk", bufs=3) as wrk, \
             tc.tile_pool(name="smp", bufs=3) as smp, \
             tc.tile_pool(name="ftp", bufs=2) as ftp, \
             tc.tile_pool(name="fgdp", bufs=9) as fgdp, \
             tc.tile_pool(name="epool", bufs=9) as epool, \
             tc.tile_pool(name="atp", bufs=9) as atp, \
             tc.tile_pool(name="updp", bufs=3) as updp:

            # ---- phase B/C: emb build + low-rank field^T projection ----
            # pos_table rows fetched by a banded one-hot matmul against a
            # linearly-DMA'd bf16 table (no SWDGE); then
            # field^T = (A @ (B^T emb))^T: M = B^T emb [RK, D], fT = M^T A^T
            with tc.tile_pool(name="psA", bufs=1, space="PSUM") as psA, \
                 tc.tile_pool(name="psM", bufs=2, space="PSUM") as psMp, \
                 tc.tile_pool(name="ptp", bufs=1) as ptp, \
                 tc.tile_pool(name="ohp", bufs=4) as ohp:
                accM = psA.tile([RK, D], F32, name="accM", tag="accM")
                # pos_table tiles: DMA issued lazily in consumption order
                pt_sb = [None] * (MAXLEN // P)

                def pt_tile(k):
                    if pt_sb[k] is None:
                        t = ptp.tile([P, D], BF16, name=f"pt{k}",
                                     tag=f"pt{k}")
                        nc.sync.dma_start(t[:], pt_d[ts(k, P), :])
                        pt_sb[k] = t
                    return pt_sb[k]

                for nt in range(NT):
                    # one-hot gather: ptg = onehot(idx)^T @ pos_table
                    ps_pt = psMp.tile([P, D], F32, name="ps_pt", tag="ps_pt")
                    bl = pt_bands[nt]
                    for bi, k in enumerate(bl):
                        oh = ohp.tile([P, P], BF16, name="oh", tag="oh")
                        nc.vector.tensor_scalar(
                            out=oh[:], in0=idxb[:, ts(nt, P)],
                            scalar1=giota_all[:, k:k + 1], scalar2=None,
                            op0=ALU.is_equal)
                        nc.tensor.matmul(ps_pt[:], lhsT=oh[:],
                                         rhs=pt_tile(k)[:],
                                         start=(bi == 0),
                                         stop=(bi == len(bl) - 1))
                    # emb = emb0_sorted + pos_table[idx]  (sorted order);
                    # the DVE add writes f32r (rounding producer)
                    embt = smp.tile([P, D], F32, name="embt", tag="embt")
                    nc.sync.dma_start(embt[:], emb0_d[ts(nt, P), :])
                    embr = embp.tile([P, D], F32R, name="embr", tag="embr")
                    nc.vector.tensor_add(embr[:], embt[:], ps_pt[:])
                    emb_sb.append(embr)
                    # accumulate M[r, d] over token tiles
                    nc.tensor.matmul(
                        accM[:],
                        lhsT=ball[:, nt * RK:(nt + 1) * RK],
                        rhs=embr[:],
                        start=(nt == 0), stop=(nt == NT - 1))
                msb = wrk.tile([RK, D], F32R, name="msb", tag="msb")
                nc.scalar.copy(msb[:], accM[:])

                at_sb = const.tile([RK, G], F32R, name="at_sb", tag="at_sb")
                nc.sync.dma_start(at_sb[:], at_d[:, :])
                # field^T tiles [128 d, 1024 g], stored f32r (PE-ready);
                # elementwise readers use .bitcast(F32) views
                atr = at_sb[:]
                fT = []
                for db in range(DB):
                    t = ftp.tile([P, G], F32R, name=f"fT{db}", tag=f"fT{db}")
                    for gc in range(2):
                        pm = psMp.tile([P, 512], F32, name="pm", tag="pm")
                        nc.tensor.matmul(pm[:], lhsT=msb[:, ts(db, P)],
                                         rhs=atr[:, ts(gc, 512)],
                                         start=True, stop=True)
                        nc.scalar.copy(t[:, ts(gc, 512)], pm[:])
                    fT.append(t)

            # ---- phase D: evolution ----
            # one shared PSUM ring: [128, 1024] f32 tiles (2 banks x 4 bufs
            # = all 8 banks); other layouts use bitcast views of it
            with tc.tile_pool(name="psB", bufs=4, space="PSUM") as psB, \
                 tc.tile_pool(name="dzp", bufs=10) as dzp:

                for step in range(dbg_steps):
                    # 1. field in [g, d] layout, fp8 hb-PAIR tiles
                    #    [128, 2, 512] (DoubleRow lhsT for inter; inter
                    #    enters the update scaled by dt*beta)
                    fgd = []
                    for pr in range(GB // 2):
                        pst = psB.tile([P, G], F32, name="big", tag="big")
                        pr_r = pst[:].bitcast(F32R)
                        for j in range(2):
                            for db in range(DB):
                                nc.tensor.transpose(
                                    pr_r[:, j * 512 + db * P:
                                         j * 512 + (db + 1) * P],
                                    fT[db][:, ts(2 * pr + j, P)],
                                    identr[:])
                        t = fgdp.tile([P, 2, 512], FP8, name="fgd",
                                      tag="fgd")
                        nc.scalar.copy(
                            t[:], pr_r.rearrange("p (b c) -> p b c", b=2))
                        fgd.append(t)

                    # 1b. lap term t3 = f + dt*alpha*lap(f)
                    #     = (1-2*adt)*f + adt*f_up + adt*f_dn, three fused
                    #     passes; depends only on fT, so it overlaps scores
                    t3s = []
                    for db in range(DB):
                        f = fT[db][:].bitcast(F32)
                        adt = adt_col[db][:]
                        z = updp.tile([P, G], F32, name="upd", tag="upd",
                                      bufs=5)
                        nc.gpsimd.tensor_scalar_mul(z[:], f, c0_col[db][:])
                        # += adt*f_up  (edge: f_up[G-1] = f[G-1])
                        nc.vector.scalar_tensor_tensor(
                            out=z[:, 0:G - 1], in0=f[:, 1:G], scalar=adt,
                            in1=z[:, 0:G - 1], op0=ALU.mult, op1=ALU.add)
                        nc.vector.scalar_tensor_tensor(
                            out=z[:, G - 1:G], in0=f[:, G - 1:G], scalar=adt,
                            in1=z[:, G - 1:G], op0=ALU.mult, op1=ALU.add)
                        # += adt*f_dn  (edge: f_dn[0] = f[0])
                        nc.vector.scalar_tensor_tensor(
                            out=z[:, 1:G], in0=f[:, 0:G - 1], scalar=adt,
                            in1=z[:, 1:G], op0=ALU.mult, op1=ALU.add)
                        nc.vector.scalar_tensor_tensor(
                            out=z[:, 0:1], in0=f[:, 0:1], scalar=adt,
                            in1=z[:, 0:1], op0=ALU.mult, op1=ALU.add)
                        t3s.append(z)

                    # 2. scores + softmax per grid block (attn in bf16)
                    attn = []
                    for gb in range(GB):
                        ps_s = psB.tile([P, G], F32, name="big", tag="big")
                        for db in range(DB):
                            for hc in range(2):
                                nc.tensor.matmul(
                                    ps_s[:, ts(hc, 512)],
                                    lhsT=fT[db][:, ts(gb, P)],
                                    rhs=fT[db][:, ts(hc, 512)],
                                    start=(db == 0), stop=(db == DB - 1))
                        # stabilizer: rowmax over every 4th column; S rows
                        # are smooth in h (field correlation ~200 grid pts),
                        # so the bias is at most ~1 exp-unit below the true
                        # max -- harmless (E <= e)
                        m_col = colp.tile([P, 1], F32, name="col", tag="col")
                        nc.vector.tensor_reduce(m_col[:], ps_s[:, 0:G:4],
                                                axis=AX.X, op=ALU.max)
                        negm = colp.tile([P, 1], F32, name="col", tag="col")
                        nc.vector.tensor_scalar_mul(negm[:], m_col[:],
                                                    -scale)
                        zcol = colp.tile([P, 1], F32, name="col", tag="col")
                        et = epool.tile([P, G], BF16, name="E", tag="E")
                        nc.scalar.activation(et[:], ps_s[:], ACTF.Exp,
                                             bias=negm[:], scale=scale,
                                             accum_out=zcol[:])
                        zinv = colp.tile([P, 1], F32, name="col", tag="col")
                        nc.vector.reciprocal(zinv[:], zcol[:])
                        # normalization is folded into the transpose as a
                        # diag(zinv) matmul: build the diagonal now
                        dz = dzp.tile([P, P], BF16, name="dz", tag="dz")
                        nc.vector.tensor_scalar_mul(dz[:], identb[:],
                                                    zinv[:])
                        attn.append((et, dz))

                    # 3. attn^T with the softmax normalization fused in:
                    #    attnT[h, g] = E[g, h] * zinv[g] = (E_chunk.T @
                    #    diag(zinv)) -- a regular bf16 matmul per block,
                    #    same PE cost as a transpose; fp32 psum -> fp8 copy
                    attnT = []
                    for pr in range(GB // 2):
                        at = atp.tile([P, 2, G], FP8, name="aT", tag="aT")
                        attnT.append(at)
                    for hb in range(GB):
                        pst = psB.tile([P, G], F32, name="big", tag="big")
                        for gb in range(GB):
                            et, dz = attn[gb]
                            nc.tensor.matmul(
                                pst[:, ts(gb, P)],
                                lhsT=et[:, ts(hb, P)], rhs=dz[:],
                                start=True, stop=True)
                        if hb % 2 == 0:
                            nc.scalar.copy(attnT[hb // 2][:, hb % 2, :],
                                           pst[:])
                        else:
                            nc.vector.tensor_copy(
                                attnT[hb // 2][:, hb % 2, :], pst[:])


                    # 4. inter^T (fp8 DoubleRow: 256 contraction rows/mm)
                    #    + field update per embed block
                    fT_new = []
                    for db in range(DB):
                        ps_i = psB.tile([P, G], F32, name="big", tag="big")
                        for pr in range(GB // 2):
                            for gc in range(2):
                                nc.tensor.matmul(
                                    ps_i[:, ts(gc, 512)],
                                    lhsT=fgd[pr][:, :, ts(db, P)],
                                    rhs=attnT[pr][:, :, ts(gc, 512)],
                                    start=(pr == 0), stop=(pr == GB // 2 - 1),
                                    perf_mode=DR)
                        # f_new = beta*dt*inter + t3  (written f32r-rounded)
                        fn = ftp.tile([P, G], F32R, name=f"fT{db}",
                                      tag=f"fT{db}")
                        nc.vector.scalar_tensor_tensor(
                            out=fn[:], in0=ps_i[:], scalar=beta_dt,
                            in1=t3s[db][:], op0=ALU.mult, op1=ALU.add)
                        fT_new.append(fn)
                    fT = fT_new

                # ---- phase E: final field -> [g, d] f32r tiles in SBUF ----
                for pr in range(GB // 2):
                    pst = psB.tile([P, G], F32, name="big", tag="big")
                    pr_r = pst[:].bitcast(F32R)
                    for j in range(2):
                        gb = 2 * pr + j
                        for db in range(DB):
                            nc.tensor.transpose(
                                pr_r[:, j * 512 + db * P:
                                     j * 512 + (db + 1) * P],
                                fT[db][:, ts(gb, P)], identr[:])
                    t = ffinp.tile([P, 2, 512], F32R, name=f"ffin{pr}",
                                   tag=f"ffin{pr}")
                    nc.scalar.copy(
                        t[:], pr_r.rearrange("p (b c) -> p b c", b=2))
                    ffin.append(t)

        # ======== phase F scope (evolution SBUF freed) ========
        inv_d = 1.0 / D
        with tc.tile_pool(name="wtp", bufs=1) as wtp, \
             tc.tile_pool(name="lnp", bufs=4) as lnp, \
             tc.tile_pool(name="psF", bufs=2, space="PSUM") as psF, \
             tc.tile_pool(name="psG", bufs=2, space="PSUM") as psG:
            # output-projection weights, f32r via bitcast (no copy needed)
            w_sb = []
            for db in range(DB):
                w = wtp.tile([P, D], F32R, name=f"wst{db}", tag=f"wst{db}")
                nc.sync.dma_start(w[:], wout_d[ts(db, P), :])
                w_sb.append(w[:])
            # NEGATED interpolation matrix -W^T[g, n] = min(|u_n-g|,1)-1 =
            # -relu(1 - |u_n - g|): exactly minus the (1-w, w) linear-interp
            # weights (stage_a compensates with a negated residual op).
            # Only the banded [128, 128] blocks are nonzero (sorted tokens)
            u_bcast = wtp.tile([P, N], F32, name="u_bcast", tag="u_bcast")
            nc.sync.dma_start(u_bcast[:], urow_d[:, :].to_broadcast((P, N)))
            wblk = {}
            for nt in range(NT if dbg_do_f else 0):
                for gb in bands[nt]:
                    q = wtp.tile([P, P], F32, name="wq", tag="wq", bufs=3)
                    nc.vector.tensor_scalar_sub(q[:],
                                                u_bcast[:, ts(nt, P)],
                                                giota_all[:, gb:gb + 1])
                    nc.scalar.activation(q[:], q[:], ACTF.Abs)
                    wt = wtp.tile([P, P], F32R, name=f"wb{nt}_{gb}",
                                  tag=f"wb{nt}_{gb}")
                    nc.vector.tensor_scalar(
                        out=wt[:], in0=q[:],
                        scalar1=1.0, scalar2=1.0,
                        op0=ALU.min, op1=ALU.subtract)
                    wblk[(nt, gb)] = wt

            def ln_stats(src, ssum, ssq_engine):
                """mean/rstd/bias from row sum + sum of squares.
                Returns (rstd, nb) col APs: norm = src*rstd + nb."""
                ssq = colp.tile([P, 1], F32, name="col", tag="col")
                scr = lnp.tile([P, D], F32, name="scr", tag="scr")
                if ssq_engine == "act":
                    nc.scalar.activation(scr[:], src[:], ACTF.Square,
                                         accum_out=ssq[:])
                else:
                    nc.vector.scalar_tensor_tensor(
                        out=scr[:], in0=src[:], scalar=1.0,
                        in1=src[:], op0=ALU.mult, op1=ALU.mult,
                        accum_out=ssq[:])
                nmean = colp.tile([P, 1], F32, name="col", tag="col")
                nc.gpsimd.tensor_scalar_mul(nmean[:], ssum[:], -inv_d)
                msq = colp.tile([P, 1], F32, name="col", tag="col")
                nc.gpsimd.tensor_mul(msq[:], nmean[:], nmean[:])
                v = colp.tile([P, 1], F32, name="col", tag="col")
                nc.vector.scalar_tensor_tensor(
                    out=v[:], in0=ssq[:], scalar=inv_d, in1=msq[:],
                    op0=ALU.mult, op1=ALU.subtract)
                sstd = colp.tile([P, 1], F32, name="col", tag="col")
                nc.scalar.activation(sstd[:], v[:], ACTF.Sqrt,
                                     bias=eps_col[:])
                rstd = colp.tile([P, 1], F32, name="col", tag="col")
                nc.vector.reciprocal(rstd[:], sstd[:])
                nb = colp.tile([P, 1], F32, name="col", tag="col")
                nc.gpsimd.tensor_mul(nb[:], nmean[:], rstd[:])
                return rstd, nb

            def stage_a(nt):
                """sample + residual -> enh (f32r).

                With trivial LN1, LN1's rstd cancels through the (host-
                folded) W+I projection because LN2 is invariant to a
                per-row positive scale: enh = xx - mean(xx) suffices."""
                ema = emb_sb[nt][:].bitcast(F32)
                # sampled = W^T.T @ field   [128 tok, 512 d]
                # (band-sparse: sorted tokens touch only bands[nt] blocks)
                ps_sm = psF.tile([P, 512], F32, name="smp", tag="smp",
                                 bufs=4)
                bl = bands[nt]
                for bi, gb in enumerate(bl):
                    nc.tensor.matmul(ps_sm[:],
                                     lhsT=wblk[(nt, gb)][:],
                                     rhs=ffin[gb // 2][:, gb % 2, :],
                                     start=(bi == 0),
                                     stop=(bi == len(bl) - 1))
                # x = sampled + emb ; accum row-sum for LN1 mean
                # (wblk holds -W, so psum is -sampled: negate here)
                xx = lnp.tile([P, D], F32, name="xx", tag="xx", bufs=8)
                ssum = colp.tile([P, 1], F32, name="col", tag="col")
                nc.vector.scalar_tensor_tensor(
                    out=xx[:], in0=ps_sm[:], scalar=-1.0,
                    in1=ema, op0=ALU.mult, op1=ALU.add,
                    accum_out=ssum[:])
                enh = lnp.tile([P, D], F32R, name="enh", tag="enh", bufs=3)
                if ln1_trivial:
                    nmean = colp.tile([P, 1], F32, name="col", tag="col")
                    nc.gpsimd.tensor_scalar_mul(nmean[:], ssum[:], -inv_d)
                    nc.scalar.activation(enh[:], xx[:], ACTF.Identity,
                                         bias=nmean[:])
                else:
                    rstd, nb = ln_stats(xx, ssum, "act")
                    nc.scalar.activation(enh[:], xx[:], ACTF.Identity,
                                         scale=rstd[:], bias=nb[:])
                    enhf = enh[:].bitcast(F32)
                    nc.gpsimd.tensor_mul(enhf, enhf, g1row[:])
                    nc.gpsimd.tensor_add(enhf, enhf, b1row[:])
                return enh

            def stage_b(nt, enh):
                """out_proj (W+I folded on host: residual included) + LN2
                -> DRAM."""
                ps_e = psG.tile([P, 512], F32R, name="sm", tag="sm")
                for db in range(DB):
                    nc.tensor.transpose(ps_e[:, ts(db, P)],
                                        enh[:, ts(db, P)], identr[:])
                enhT = lnp.tile([P, D], F32R, name="enhT", tag="enhT")
                nc.vector.tensor_copy(enhT[:], ps_e[:])
                ps_o = psF.tile([P, 512], F32, name="big", tag="big",
                                bufs=2)
                for db in range(DB):
                    nc.tensor.matmul(ps_o[:],
                                     lhsT=enhT[:, ts(db, P)],
                                     rhs=w_sb[db][:],
                                     start=(db == 0), stop=(db == DB - 1))
                # y already includes the residual via W+I (+ b_out) ; LN2
                yy = lnp.tile([P, D], F32, name="yy", tag="xx", bufs=8)
                ysum = colp.tile([P, 1], F32, name="col", tag="col")
                if bout_trivial:
                    nc.vector.tensor_scalar(
                        out=yy[:], in0=ps_o[:], scalar1=1.0, scalar2=0.0,
                        op0=ALU.mult, op1=ALU.add, accum_out=ysum[:])
                else:
                    nc.gpsimd.tensor_add(yy[:], ps_o[:], boutrow[:])
                    nc.vector.tensor_reduce(ysum[:], yy[:], axis=AX.X,
                                            op=ALU.add)
                rstd2, nb2 = ln_stats(yy, ysum, "dve")
                res = lnp.tile([P, D], F32, name="res", tag="res")
                nc.scalar.activation(res[:], yy[:], ACTF.Identity,
                                     scale=rstd2[:], bias=nb2[:])
                if not ln2_trivial:
                    nc.gpsimd.tensor_mul(res[:], res[:], g2row[:])
                    nc.gpsimd.tensor_add(res[:], res[:], b2row[:])
                # rows stay in sorted-token order; host unpermutes
                nc.sync.dma_start(out_d[ts(nt, P), :], res[:])

            # software-pipelined with skew 2: PE runs sampled-matmuls of
            # nt+1/nt+2 while the LN chain of nt completes
            SKEW = 3
            nF = NT if dbg_do_f else 0
            pend = []
            for nt in range(nF):
                pend.append((nt, stage_a(nt)))
                if len(pend) > SKEW:
                    j, e = pend.pop(0)
                    stage_b(j, e)
            for j, e in pend:
                stage_b(j, e)

    nc.compile()
    return nc


def host_prep(embeddings, positions, grid_points, pos_table, sigma, alpha,
              beta, dt, ln1_g, ln1_b, ln2_g, ln2_b, w_out, b_out):
    """Host-side prep: derived index tensors + per-core input maps."""
    embeddings = np.asarray(embeddings, np.float32)
    positions = np.asarray(positions, np.float32)
    grid_points = np.asarray(grid_points, np.float32)
    pos_table = np.ascontiguousarray(np.asarray(pos_table, np.float32))
    alpha = np.asarray(alpha, np.float32)
    # residual fold: out+enh = enh @ (W + I)
    w_out = np.ascontiguousarray(np.asarray(w_out, np.float32)
                                 + np.eye(D, dtype=np.float32))
    b_out = np.asarray(b_out, np.float32)
    sigma = np.float32(np.asarray(sigma))
    beta = np.float32(np.asarray(beta))
    dt = np.float32(np.asarray(dt))
    ln1_g = np.asarray(ln1_g, np.float32)
    ln1_b = np.asarray(ln1_b, np.float32)
    ln2_g = np.asarray(ln2_g, np.float32)
    ln2_b = np.asarray(ln2_b, np.float32)

    c_exp = float(-(np.float32(1.0) / (np.float32(2.0) * sigma * sigma)))
    scale = float(np.float32(1.0) / np.sqrt(np.float32(D)))
    beta_dt = float(beta * dt)
    alphadt = np.ascontiguousarray((dt * alpha).astype(np.float32)
                                   .reshape(D, 1))

    # Chebyshev factorization of the RBF kernel: K(u,p) = L(u) Kc L(p)^T,
    # exact to ~1e-15 at RK nodes (kernel is entire, sigma=0.2 wide)
    kq = np.arange(RK)
    tn = 0.5 + 0.5 * np.cos((2 * kq + 1) * np.pi / (2 * RK))
    bw = np.empty(RK)
    for j in range(RK):
        bw[j] = 1.0 / np.prod(tn[j] - np.delete(tn, j))

    def lagrange(x):
        diff = x[:, None] - tn[None, :]
        hit = np.isclose(diff, 0.0, atol=1e-14)
        diff = np.where(hit, 1.0, diff)
        num = bw[None, :] / diff
        L = num / num.sum(1, keepdims=True)
        rows = hit.any(1)
        L[rows] = hit[rows].astype(np.float64)
        return L

    ln1_trivial = bool(np.all(ln1_g == 1.0) and np.all(ln1_b == 0.0))
    ln2_trivial = bool(np.all(ln2_g == 1.0) and np.all(ln2_b == 0.0))
    bout_trivial = bool(np.all(b_out == 0.0))

    import ml_dtypes

    def to_f32r(x):
        xb = np.asarray(x, np.float32).view(np.uint32)
        sh = np.uint32(13)
        r = ((xb >> sh) + ((xb >> np.uint32(12)) & np.uint32(1))) << sh
        return r.view(np.float32)

    pt_bf16 = np.ascontiguousarray(pos_table.astype(ml_dtypes.bfloat16))
    giota = np.arange(MAXLEN, dtype=np.float32).reshape(MAXLEN, 1)
    in_maps = []
    all_bands = []
    all_pt_bands = []
    all_orders = []
    for c in range(NCORES):
        pos_n = positions[c, :, 0]                     # [N] fp32 (natural)
        u_n = pos_n * np.float32(G - 1)
        order = np.argsort(u_n, kind="stable").astype(np.int32)
        all_orders.append(order)
        pos = pos_n[order]                             # sorted token order
        u = u_n[order]
        idx = np.clip(np.rint(pos * np.float32(MAXLEN - 1)).astype(np.int32),
                      0, MAXLEN - 1)
        # grid blocks each sorted token tile touches (i0..i0+1 support)
        i0 = np.clip(np.floor(u).astype(np.int64), 0, G - 1)
        ihi = np.minimum(i0 + 1, G - 1)
        bands = []
        for nt in range(NT):
            lo = int(i0[nt * P:(nt + 1) * P].min()) // P
            hi = int(ihi[nt * P:(nt + 1) * P].max()) // P
            bands.append(tuple(range(lo, hi + 1)))
        all_bands.append(tuple(bands))
        ptb = []
        for nt in range(NT):
            lo = int(idx[nt * P:(nt + 1) * P].min()) // P
            hi = int(idx[nt * P:(nt + 1) * P].max()) // P
            ptb.append(tuple(range(lo, hi + 1)))
        all_pt_bands.append(tuple(ptb))
        u_g = grid_points[c, :, 0].astype(np.float64)
        amatT = (np.exp(-(tn[:, None] - tn[None, :]) ** 2
                        / (2.0 * float(sigma) ** 2)) @ lagrange(u_g).T)
        bmat = lagrange(pos.astype(np.float64))
        m = {
            "emb0": np.ascontiguousarray(embeddings[c][order]),
            "pt": pt_bf16,
            "bmat": np.ascontiguousarray(to_f32r(
                bmat.astype(np.float32).reshape(NT, P, RK)
                .transpose(1, 0, 2).reshape(P, NT * RK))),
            "amatT": np.ascontiguousarray(to_f32r(amatT)),
            "idxrow": np.ascontiguousarray(
                idx.astype(np.float32).reshape(1, N)),
            "urow": np.ascontiguousarray(u.reshape(1, N)),
            "giota": giota,
            "alphadt": alphadt,
            "wout": to_f32r(w_out),
        }
        if not ln1_trivial:
            m["ln1g"] = np.ascontiguousarray(ln1_g.reshape(1, D))
            m["ln1b"] = np.ascontiguousarray(ln1_b.reshape(1, D))
        if not ln2_trivial:
            m["ln2g"] = np.ascontiguousarray(ln2_g.reshape(1, D))
            m["ln2b"] = np.ascontiguousarray(ln2_b.reshape(1, D))
        if not bout_trivial:
            m["bout"] = np.ascontiguousarray(b_out.reshape(1, D))
        in_maps.append(m)

    # SPMD: one program for all cores -> per-tile band = union over cores
    bands = tuple(
        tuple(range(min(b[nt][0] for b in all_bands),
                    max(b[nt][-1] for b in all_bands) + 1))
        for nt in range(NT))
    pt_bands = tuple(
        tuple(range(min(b[nt][0] for b in all_pt_bands),
                    max(b[nt][-1] for b in all_pt_bands) + 1))
        for nt in range(NT))
    build_key = (c_exp, scale, beta_dt, ln1_trivial, ln2_trivial,
                 bout_trivial, bands, pt_bands)
    return in_maps, build_key, all_orders


_NC_CACHE = {}


def kernel(**inputs):
    in_maps, build_key, orders = host_prep(**inputs)
    if build_key not in _NC_CACHE:
        _NC_CACHE[build_key] = build_nc(*build_key)
    nc = _NC_CACHE[build_key]
    res = run_bass_kernel_spmd(nc, in_maps, list(range(NCORES)))
    out = np.empty((NCORES, N, D), np.float32)
    for i in range(NCORES):
        out[i, orders[i], :] = res.results[i]["out"]
    return out



# revision 39
# speedup vs baseline: 1.0726x; 1.0133x over previous
"""Trainium2 Bass kernel for nn_EnhancedTFNLayer.

Sharding: data-parallel over batch — B=8 batch elements, one per NeuronCore.
Each core runs the full pipeline for its batch element:

  emb   = embeddings + pos_table[idx]                       (indirect DMA gather)
  K^T   = exp(-(grid_g - pos_n)^2 / (2 sigma^2))            [n, g] layout
  f^T   = sum_n emb[n, d] K^T[n, g]                         (PE, accumulated in PSUM)
  4x evolution step:
      f_gd   = transpose(f^T)                               (PE transposes)
      scores = f^T.T @ f^T  (symmetric, per 128-row block)  (PE)
      E      = exp(scale*s - scale*rowmax)   + rowsum Z     (ACT, fused accum)
      attn   = E / Z                                        (GpSimd, in place)
      attn^T = transpose(attn)                              (PE transposes)
      inter^T= f_gd.T @ attn^T                              (PE)
      f^T   += dt*alpha*lap(f^T) + dt*beta*inter^T          (DVE fused passes)
  sampled = W^T.T @ field  with W[n,g] = relu(1-|u_n-g|)    (PE; exact lerp)
  LN1 -> out_proj (PE) -> LN2 -> out (sorted order; host unpermutes)

Tokens are processed sorted by position: the interpolation matrix W then
becomes block-banded (1-2 grid blocks per 128-token tile instead of 8).
Matmul operands are float32r (fp32 rounded to 11-bit mantissa, 4x PE rate
vs plain fp32); attention internals (attn, attn^T, field in [g,d]) are
bf16 since `inter` enters the field update scaled by dt*beta=0.01.
Host precomputes gather indices, sorted u values, and the sort
permutation (exact fp32 semantics of the reference).
"""

import math
from contextlib import ExitStack

import numpy as np

import concourse.bacc as bacc
import concourse.bass as bass
import concourse.tile as tile
from concourse import mybir
from concourse.bass_utils import run_bass_kernel_spmd
from concourse.masks import make_identity
from concourse.tile_rust import add_dep_helper

P = 128
N = 2048          # tokens
D = 512           # embed dim
G = 1024          # grid points
MAXLEN = 2048     # pos table rows
NT = N // P       # 16 token tiles
DB = D // P       # 4 embed blocks
GB = G // P       # 8 grid blocks
RK = 64           # Chebyshev rank of the RBF kernel factorization
STEPS = 4
LN_EPS = 1e-5
NCORES = 8

F32 = mybir.dt.float32
F32R = mybir.dt.float32r
BF16 = mybir.dt.bfloat16
FP8 = mybir.dt.float8e4
DR = mybir.MatmulPerfMode.DoubleRow
I32 = mybir.dt.int32
AX = mybir.AxisListType
ALU = mybir.AluOpType
ACTF = mybir.ActivationFunctionType


def _r(ap):
    """Bitcast fp32 AP to float32r for full-rate PE matmul."""
    return ap.bitcast(F32R)


def ts(i, size):
    return slice(i * size, (i + 1) * size)

def build_nc(c_exp, scale, beta_dt, ln1_trivial=True, ln2_trivial=True,
             bout_trivial=True, bands=None, pt_bands=None, dbg_steps=STEPS,
             dbg_do_f=True):
    """Build and compile the per-core Bass program.

    c_exp   : -1/(2 sigma^2) as float (fp32-rounded)
    scale   : 1/sqrt(D) as float
    beta_dt : beta*dt as float
    bands   : per token-tile tuple of grid blocks its sorted positions
              touch (sampling matmul band sparsity)
    pt_bands: per token-tile tuple of 128-row pos_table blocks its
              (sorted) idx values touch (one-hot gather matmul)
    """
    if bands is None:
        bands = tuple(tuple(range(GB)) for _ in range(NT))
    if pt_bands is None:
        pt_bands = tuple(tuple(range(MAXLEN // P)) for _ in range(NT))
    nc = bacc.Bacc()

    # ---- I/O ----
    # emb0 rows are pre-sorted by position on the host (plain DMA load)
    emb0_d = nc.declare_dram_parameter("emb0", [N, D], F32, isOutput=False)
    # pos_table in bf16 (entries ~0.02; bf16 error is negligible)
    pt_d = nc.declare_dram_parameter("pt", [MAXLEN, D], BF16, isOutput=False)
    # low-rank RBF factorization: K = A @ B^T (Chebyshev, exact to 1e-15)
    bm_d = nc.declare_dram_parameter("bmat", [P, NT * RK], F32R,
                                     isOutput=False)
    at_d = nc.declare_dram_parameter("amatT", [RK, G], F32R, isOutput=False)
    idxrow_d = nc.declare_dram_parameter("idxrow", [1, N], F32,
                                         isOutput=False)
    urow_d = nc.declare_dram_parameter("urow", [1, N], F32, isOutput=False)
    giota_d = nc.declare_dram_parameter("giota", [MAXLEN, 1], F32,
                                       isOutput=False)
    adt_d = nc.declare_dram_parameter("alphadt", [D, 1], F32, isOutput=False)
    wout_d = nc.declare_dram_parameter("wout", [D, D], F32R, isOutput=False)
    if not ln1_trivial:
        ln1g_d = nc.declare_dram_parameter("ln1g", [1, D], F32, isOutput=False)
        ln1b_d = nc.declare_dram_parameter("ln1b", [1, D], F32, isOutput=False)
    if not ln2_trivial:
        ln2g_d = nc.declare_dram_parameter("ln2g", [1, D], F32, isOutput=False)
        ln2b_d = nc.declare_dram_parameter("ln2b", [1, D], F32, isOutput=False)
    if not bout_trivial:
        bout_d = nc.declare_dram_parameter("bout", [1, D], F32, isOutput=False)
    out_d = nc.declare_dram_parameter("out", [N, D], F32, isOutput=True)


    with tile.TileContext(nc) as tc:
      with tc.tile_pool(name="const", bufs=1) as const, \
           tc.tile_pool(name="colp", bufs=48) as colp, \
           tc.tile_pool(name="ffinp", bufs=1) as ffinp, \
           tc.tile_pool(name="embp", bufs=16) as embp:
        # ---- constants (live for the whole kernel) ----
        ident = const.tile([P, P], F32, name="ident", tag="ident")
        make_identity(nc, ident[:])
        identr = const.tile([P, P], F32R, name="identr", tag="identr")
        nc.scalar.copy(identr[:], ident[:])
        identb = const.tile([P, P], BF16, name="identb", tag="identb")
        nc.scalar.copy(identb[:], ident[:])
        # critical-path loads first: idx row broadcast (one-hot pt gather),
        # B factor, row-block iotas
        idxb = const.tile([P, N], F32, name="idxb", tag="idxb")
        nc.sync.dma_start(idxb[:], idxrow_d[:, :].to_broadcast((P, N)))
        # B factor, all 16 token tiles batched: [128, nt*RK + j]
        ball = const.tile([P, NT * RK], F32R, name="ball", tag="ball")
        nc.sync.dma_start(ball[:], bm_d[:, :])
        giota_all = const.tile([P, MAXLEN // P], F32, name="giota_all",
                               tag="giota_all")
        nc.sync.dma_start(giota_all[:],
                          giota_d[:, :].rearrange("(a b) c -> b (a c)", b=P))
        adt_col = []
        for db in range(DB):
            a = const.tile([P, 1], F32, name=f"adt{db}", tag=f"adt{db}")
            nc.sync.dma_start(a[:], adt_d[ts(db, P), :])
            adt_col.append(a)
        c0_col = []
        for db in range(DB):
            c = const.tile([P, 1], F32, name=f"c0_{db}", tag=f"c0_{db}")
            nc.vector.tensor_scalar(out=c[:], in0=adt_col[db][:],
                                    scalar1=-2.0, scalar2=1.0,
                                    op0=ALU.mult, op1=ALU.add)
            c0_col.append(c)
        eps_col = const.tile([P, 1], F32, name="eps", tag="eps")
        nc.vector.memset(eps_col[:], LN_EPS)
        if not ln1_trivial:
            g1row = const.tile([P, D], F32, name="g1row", tag="g1row")
            nc.sync.dma_start(g1row[:], ln1g_d[:, :].to_broadcast((P, D)))
            b1row = const.tile([P, D], F32, name="b1row", tag="b1row")
            nc.sync.dma_start(b1row[:], ln1b_d[:, :].to_broadcast((P, D)))
        if not ln2_trivial:
            g2row = const.tile([P, D], F32, name="g2row", tag="g2row")
            nc.sync.dma_start(g2row[:], ln2g_d[:, :].to_broadcast((P, D)))
            b2row = const.tile([P, D], F32, name="b2row", tag="b2row")
            nc.sync.dma_start(b2row[:], ln2b_d[:, :].to_broadcast((P, D)))
        if not bout_trivial:
            boutrow = const.tile([P, D], F32, name="boutrow", tag="boutrow")
            nc.sync.dma_start(boutrow[:], bout_d[:, :].to_broadcast((P, D)))

        emb_sb = []  # resident f32r emb tiles (sorted order)
        ffin = []   # final field [g, d] tiles, f32r, kept for sampling

        # ======== evolution scope: pools freed before phase F ========
        with tc.tile_pool(name="wrk", bufs=3) as wrk, \
             tc.tile_pool(name="smp", bufs=3) as smp, \
             tc.tile_pool(name="ftp", bufs=2) as ftp, \
             tc.tile_pool(name="fgdp", bufs=9) as fgdp, \
             tc.tile_pool(name="epool", bufs=9) as epool, \
             tc.tile_pool(name="atp", bufs=9) as atp, \
             tc.tile_pool(name="updp", bufs=3) as updp:

            # ---- phase B/C: emb build + low-rank field^T projection ----
            # pos_table rows fetched by a banded one-hot matmul against a
            # linearly-DMA'd bf16 table (no SWDGE); then
            # field^T = (A @ (B^T emb))^T: M = B^T emb [RK, D], fT = M^T A^T
            with tc.tile_pool(name="psA", bufs=1, space="PSUM") as psA, \
                 tc.tile_pool(name="psM", bufs=2, space="PSUM") as psMp, \
                 tc.tile_pool(name="ptp", bufs=1) as ptp, \
                 tc.tile_pool(name="ohp", bufs=4) as ohp:
                accM = psA.tile([RK, D], F32, name="accM", tag="accM")
                # pos_table tiles: DMA issued lazily in consumption order
                pt_sb = [None] * (MAXLEN // P)

                def pt_tile(k):
                    if pt_sb[k] is None:
                        t = ptp.tile([P, D], BF16, name=f"pt{k}",
                                     tag=f"pt{k}")
                        nc.sync.dma_start(t[:], pt_d[ts(k, P), :])
                        pt_sb[k] = t
                    return pt_sb[k]

                for nt in range(NT):
                    # one-hot gather: ptg = onehot(idx)^T @ pos_table
                    ps_pt = psMp.tile([P, D], F32, name="ps_pt", tag="ps_pt")
                    bl = pt_bands[nt]
                    for bi, k in enumerate(bl):
                        oh = ohp.tile([P, P], BF16, name="oh", tag="oh")
                        nc.vector.tensor_scalar(
                            out=oh[:], in0=idxb[:, ts(nt, P)],
                            scalar1=giota_all[:, k:k + 1], scalar2=None,
                            op0=ALU.is_equal)
                        nc.tensor.matmul(ps_pt[:], lhsT=oh[:],
                                         rhs=pt_tile(k)[:],
                                         start=(bi == 0),
                                         stop=(bi == len(bl) - 1))
                    # emb = emb0_sorted + pos_table[idx]  (sorted order);
                    # the DVE add writes f32r (rounding producer)
                    embt = smp.tile([P, D], F32, name="embt", tag="embt")
                    nc.sync.dma_start(embt[:], emb0_d[ts(nt, P), :])
                    embr = embp.tile([P, D], F32R, name="embr", tag="embr")
                    nc.vector.tensor_add(embr[:], embt[:], ps_pt[:])
                    emb_sb.append(embr)
                    # accumulate M[r, d] over token tiles
                    nc.tensor.matmul(
                        accM[:],
                        lhsT=ball[:, nt * RK:(nt + 1) * RK],
                        rhs=embr[:],
                        start=(nt == 0), stop=(nt == NT - 1))
                msb = wrk.tile([RK, D], F32R, name="msb", tag="msb")
                nc.scalar.copy(msb[:], accM[:])

                at_sb = const.tile([RK, G], F32R, name="at_sb", tag="at_sb")
                nc.sync.dma_start(at_sb[:], at_d[:, :])
                # field^T tiles [128 d, 1024 g], stored f32r (PE-ready);
                # elementwise readers use .bitcast(F32) views
                atr = at_sb[:]
                fT = []
                for db in range(DB):
                    t = ftp.tile([P, G], F32R, name=f"fT{db}", tag=f"fT{db}")
                    for gc in range(2):
                        pm = psMp.tile([P, 512], F32, name="pm", tag="pm")
                        nc.tensor.matmul(pm[:], lhsT=msb[:, ts(db, P)],
                                         rhs=atr[:, ts(gc, 512)],
                                         start=True, stop=True)
                        nc.scalar.copy(t[:, ts(gc, 512)], pm[:])
                    fT.append(t)

            # ---- phase D: evolution ----
            # one shared PSUM ring: [128, 1024] f32 tiles (2 banks x 4 bufs
            # = all 8 banks); other layouts use bitcast views of it
            with tc.tile_pool(name="psB", bufs=4, space="PSUM") as psB, \
                 tc.tile_pool(name="dzp", bufs=10) as dzp:

                for step in range(dbg_steps):
                    # 1. field in [g, d] layout, fp8 hb-PAIR tiles
                    #    [128, 2, 512] (DoubleRow lhsT for inter; inter
                    #    enters the update scaled by dt*beta)
                    fgd = []
                    for pr in range(GB // 2):
                        pst = psB.tile([P, G], F32, name="big", tag="big")
                        pr_r = pst[:].bitcast(F32R)
                        for j in range(2):
                            for db in range(DB):
                                nc.tensor.transpose(
                                    pr_r[:, j * 512 + db * P:
                                         j * 512 + (db + 1) * P],
                                    fT[db][:, ts(2 * pr + j, P)],
                                    identr[:])
                        t = fgdp.tile([P, 2, 512], FP8, name="fgd",
                                      tag="fgd")
                        nc.scalar.copy(
                            t[:], pr_r.rearrange("p (b c) -> p b c", b=2))
                        fgd.append(t)

                    # 1b. lap term t3 = f + dt*alpha*lap(f)
                    #     = (1-2*adt)*f + adt*f_up + adt*f_dn, three fused
                    #     passes; depends only on fT, so it overlaps scores
                    t3s = []
                    for db in range(DB):
                        f = fT[db][:].bitcast(F32)
                        adt = adt_col[db][:]
                        z = updp.tile([P, G], F32, name="upd", tag="upd",
                                      bufs=5)
                        nc.gpsimd.tensor_scalar_mul(z[:], f, c0_col[db][:])
                        # += adt*f_up  (edge: f_up[G-1] = f[G-1])
                        nc.vector.scalar_tensor_tensor(
                            out=z[:, 0:G - 1], in0=f[:, 1:G], scalar=adt,
                            in1=z[:, 0:G - 1], op0=ALU.mult, op1=ALU.add)
                        nc.vector.scalar_tensor_tensor(
                            out=z[:, G - 1:G], in0=f[:, G - 1:G], scalar=adt,
                            in1=z[:, G - 1:G], op0=ALU.mult, op1=ALU.add)
                        # += adt*f_dn  (edge: f_dn[0] = f[0])
                        nc.vector.scalar_tensor_tensor(
                            out=z[:, 1:G], in0=f[:, 0:G - 1], scalar=adt,
                            in1=z[:, 1:G], op0=ALU.mult, op1=ALU.add)
                        nc.vector.scalar_tensor_tensor(
                            out=z[:, 0:1], in0=f[:, 0:1], scalar=adt,
                            in1=z[:, 0:1], op0=ALU.mult, op1=ALU.add)
                        t3s.append(z)

                    # 2. scores + softmax per grid block (attn in bf16)
                    attn = []
                    for gb in range(GB):
                        ps_s = psB.tile([P, G], F32, name="big", tag="big")
                        for db in range(DB):
                            for hc in range(2):
                                nc.tensor.matmul(
                                    ps_s[:, ts(hc, 512)],
                                    lhsT=fT[db][:, ts(gb, P)],
                                    rhs=fT[db][:, ts(hc, 512)],
                                    start=(db == 0), stop=(db == DB - 1))
                        # stabilizer: rowmax over every 4th column; S rows
                        # are smooth in h (field correlation ~200 grid pts),
                        # so the bias is at most ~1 exp-unit below the true
                        # max -- harmless (E <= e)
                        m_col = colp.tile([P, 1], F32, name="col", tag="col")
                        nc.vector.tensor_reduce(m_col[:], ps_s[:, 0:G:4],
                                                axis=AX.X, op=ALU.max)
                        negm = colp.tile([P, 1], F32, name="col", tag="col")
                        nc.vector.tensor_scalar_mul(negm[:], m_col[:],
                                                    -scale)
                        zcol = colp.tile([P, 1], F32, name="col", tag="col")
                        et = epool.tile([P, G], BF16, name="E", tag="E")
                        nc.scalar.activation(et[:], ps_s[:], ACTF.Exp,
                                             bias=negm[:], scale=scale,
                                             accum_out=zcol[:])
                        zinv = colp.tile([P, 1], F32, name="col", tag="col")
                        nc.vector.reciprocal(zinv[:], zcol[:])
                        # normalization is folded into the transpose as a
                        # diag(zinv) matmul: build the diagonal now
                        dz = dzp.tile([P, P], BF16, name="dz", tag="dz")
                        nc.vector.tensor_scalar_mul(dz[:], identb[:],
                                                    zinv[:])
                        attn.append((et, dz))

                    # 3. attn^T with the softmax normalization fused in:
                    #    attnT[h, g] = E[g, h] * zinv[g] = (E_chunk.T @
                    #    diag(zinv)) -- a regular bf16 matmul per block,
                    #    same PE cost as a transpose; fp32 psum -> fp8 copy
                    attnT = []
                    for pr in range(GB // 2):
                        at = atp.tile([P, 2, G], FP8, name="aT", tag="aT")
                        attnT.append(at)
                    for hb in range(GB):
                        pst = psB.tile([P, G], F32, name="big", tag="big")
                        for gb in range(GB):
                            et, dz = attn[gb]
                            nc.tensor.matmul(
                                pst[:, ts(gb, P)],
                                lhsT=et[:, ts(hb, P)], rhs=dz[:],
                                start=True, stop=True)
                        if hb % 2 == 0:
                            nc.scalar.copy(attnT[hb // 2][:, hb % 2, :],
                                           pst[:])
                        else:
                            nc.vector.tensor_copy(
                                attnT[hb // 2][:, hb % 2, :], pst[:])


                    # 4. inter^T (fp8 DoubleRow: 256 contraction rows/mm)
                    #    + field update per embed block
                    fT_new = []
                    for db in range(DB):
                        ps_i = psB.tile([P, G], F32, name="big", tag="big")
                        for pr in range(GB // 2):
                            for gc in range(2):
                                nc.tensor.matmul(
                                    ps_i[:, ts(gc, 512)],
                                    lhsT=fgd[pr][:, :, ts(db, P)],
                                    rhs=attnT[pr][:, :, ts(gc, 512)],
                                    start=(pr == 0), stop=(pr == GB // 2 - 1),
                                    perf_mode=DR)
                        # f_new = beta*dt*inter + t3  (written f32r-rounded)
                        fn = ftp.tile([P, G], F32R, name=f"fT{db}",
                                      tag=f"fT{db}")
                        nc.vector.scalar_tensor_tensor(
                            out=fn[:], in0=ps_i[:], scalar=beta_dt,
                            in1=t3s[db][:], op0=ALU.mult, op1=ALU.add)
                        fT_new.append(fn)
                    fT = fT_new

                # ---- phase E: final field -> [g, d] f32r tiles in SBUF ----
                for pr in range(GB // 2):
                    pst = psB.tile([P, G], F32, name="big", tag="big")
                    pr_r = pst[:].bitcast(F32R)
                    for j in range(2):
                        gb = 2 * pr + j
                        for db in range(DB):
                            nc.tensor.transpose(
                                pr_r[:, j * 512 + db * P:
                                     j * 512 + (db + 1) * P],
                                fT[db][:, ts(gb, P)], identr[:])
                    t = ffinp.tile([P, 2, 512], F32R, name=f"ffin{pr}",
                                   tag=f"ffin{pr}")
                    nc.scalar.copy(
                        t[:], pr_r.rearrange("p (b c) -> p b c", b=2))
                    ffin.append(t)

        # ======== phase F scope (evolution SBUF freed) ========
        inv_d = 1.0 / D
        with tc.tile_pool(name="wtp", bufs=1) as wtp, \
             tc.tile_pool(name="lnp", bufs=4) as lnp, \
             tc.tile_pool(name="psF", bufs=2, space="PSUM") as psF, \
             tc.tile_pool(name="psG", bufs=2, space="PSUM") as psG:
            # output-projection weights, f32r via bitcast (no copy needed)
            w_sb = []
            for db in range(DB):
                w = wtp.tile([P, D], F32R, name=f"wst{db}", tag=f"wst{db}")
                nc.sync.dma_start(w[:], wout_d[ts(db, P), :])
                w_sb.append(w[:])
            # NEGATED interpolation matrix -W^T[g, n] = min(|u_n-g|,1)-1 =
            # -relu(1 - |u_n - g|): exactly minus the (1-w, w) linear-interp
            # weights (stage_a compensates with a negated residual op).
            # Only the banded [128, 128] blocks are nonzero (sorted tokens)
            u_bcast = wtp.tile([P, N], F32, name="u_bcast", tag="u_bcast")
            nc.sync.dma_start(u_bcast[:], urow_d[:, :].to_broadcast((P, N)))
            wblk = {}
            for nt in range(NT if dbg_do_f else 0):
                for gb in bands[nt]:
                    q = wtp.tile([P, P], F32, name="wq", tag="wq", bufs=3)
                    nc.gpsimd.tensor_scalar_sub(q[:],
                                                u_bcast[:, ts(nt, P)],
                                                giota_all[:, gb:gb + 1])
                    nc.scalar.activation(q[:], q[:], ACTF.Abs)
                    wt = wtp.tile([P, P], F32R, name=f"wb{nt}_{gb}",
                                  tag=f"wb{nt}_{gb}")
                    nc.vector.tensor_scalar(
                        out=wt[:], in0=q[:],
                        scalar1=1.0, scalar2=1.0,
                        op0=ALU.min, op1=ALU.subtract)
                    wblk[(nt, gb)] = wt

            def ln_stats(src, ssum, ssq_engine):
                """mean/rstd/bias from row sum + sum of squares.
                Returns (rstd, nb) col APs: norm = src*rstd + nb."""
                ssq = colp.tile([P, 1], F32, name="col", tag="col")
                scr = lnp.tile([P, D], F32, name="scr", tag="scr")
                if ssq_engine == "act":
                    nc.scalar.activation(scr[:], src[:], ACTF.Square,
                                         accum_out=ssq[:])
                else:
                    nc.vector.scalar_tensor_tensor(
                        out=scr[:], in0=src[:], scalar=1.0,
                        in1=src[:], op0=ALU.mult, op1=ALU.mult,
                        accum_out=ssq[:])
                nmean = colp.tile([P, 1], F32, name="col", tag="col")
                nc.gpsimd.tensor_scalar_mul(nmean[:], ssum[:], -inv_d)
                msq = colp.tile([P, 1], F32, name="col", tag="col")
                nc.gpsimd.tensor_mul(msq[:], nmean[:], nmean[:])
                v = colp.tile([P, 1], F32, name="col", tag="col")
                nc.vector.scalar_tensor_tensor(
                    out=v[:], in0=ssq[:], scalar=inv_d, in1=msq[:],
                    op0=ALU.mult, op1=ALU.subtract)
                sstd = colp.tile([P, 1], F32, name="col", tag="col")
                nc.scalar.activation(sstd[:], v[:], ACTF.Sqrt,
                                     bias=eps_col[:])
                rstd = colp.tile([P, 1], F32, name="col", tag="col")
                nc.vector.reciprocal(rstd[:], sstd[:])
                nb = colp.tile([P, 1], F32, name="col", tag="col")
                nc.gpsimd.tensor_mul(nb[:], nmean[:], rstd[:])
                return rstd, nb

            def stage_a(nt):
                """sample + residual -> enh (f32r).

                With trivial LN1, LN1's rstd cancels through the (host-
                folded) W+I projection because LN2 is invariant to a
                per-row positive scale: enh = xx - mean(xx) suffices."""
                ema = emb_sb[nt][:].bitcast(F32)
                # sampled = W^T.T @ field   [128 tok, 512 d]
                # (band-sparse: sorted tokens touch only bands[nt] blocks)
                ps_sm = psF.tile([P, 512], F32, name="smp", tag="smp",
                                 bufs=4)
                bl = bands[nt]
                for bi, gb in enumerate(bl):
                    nc.tensor.matmul(ps_sm[:],
                                     lhsT=wblk[(nt, gb)][:],
                                     rhs=ffin[gb // 2][:, gb % 2, :],
                                     start=(bi == 0),
                                     stop=(bi == len(bl) - 1))
                # x = sampled + emb ; accum row-sum for LN1 mean
                # (wblk holds -W, so psum is -sampled: negate here)
                xx = lnp.tile([P, D], F32, name="xx", tag="xx", bufs=8)
                ssum = colp.tile([P, 1], F32, name="col", tag="col")
                nc.vector.scalar_tensor_tensor(
                    out=xx[:], in0=ps_sm[:], scalar=-1.0,
                    in1=ema, op0=ALU.mult, op1=ALU.add,
                    accum_out=ssum[:])
                enh = lnp.tile([P, D], F32R, name="enh", tag="enh", bufs=3)
                if ln1_trivial:
                    nmean = colp.tile([P, 1], F32, name="col", tag="col")
                    nc.gpsimd.tensor_scalar_mul(nmean[:], ssum[:], -inv_d)
                    nc.vector.tensor_scalar_add(enh[:], xx[:], nmean[:])
                else:
                    rstd, nb = ln_stats(xx, ssum, "act")
                    nc.scalar.activation(enh[:], xx[:], ACTF.Identity,
                                         scale=rstd[:], bias=nb[:])
                    enhf = enh[:].bitcast(F32)
                    nc.gpsimd.tensor_mul(enhf, enhf, g1row[:])
                    nc.gpsimd.tensor_add(enhf, enhf, b1row[:])
                return enh

            def stage_b(nt, enh):
                """out_proj (W+I folded on host: residual included) + LN2
                -> DRAM."""
                ps_e = psG.tile([P, 512], F32R, name="sm", tag="sm")
                for db in range(DB):
                    nc.tensor.transpose(ps_e[:, ts(db, P)],
                                        enh[:, ts(db, P)], identr[:])
                enhT = lnp.tile([P, D], F32R, name="enhT", tag="enhT")
                nc.scalar.copy(enhT[:], ps_e[:])
                ps_o = psF.tile([P, 512], F32, name="big", tag="big",
                                bufs=2)
                for db in range(DB):
                    nc.tensor.matmul(ps_o[:],
                                     lhsT=enhT[:, ts(db, P)],
                                     rhs=w_sb[db][:],
                                     start=(db == 0), stop=(db == DB - 1))
                # y already includes the residual via W+I (+ b_out) ; LN2
                yy = lnp.tile([P, D], F32, name="yy", tag="xx", bufs=8)
                ysum = colp.tile([P, 1], F32, name="col", tag="col")
                if bout_trivial:
                    nc.scalar.activation(yy[:], ps_o[:], ACTF.Identity,
                                         accum_out=ysum[:])
                else:
                    nc.gpsimd.tensor_add(yy[:], ps_o[:], boutrow[:])
                    nc.vector.tensor_reduce(ysum[:], yy[:], axis=AX.X,
                                            op=ALU.add)
                rstd2, nb2 = ln_stats(yy, ysum, "act")
                res = lnp.tile([P, D], F32, name="res", tag="res")
                nc.vector.tensor_scalar(
                    out=res[:], in0=yy[:], scalar1=rstd2[:], scalar2=nb2[:],
                    op0=ALU.mult, op1=ALU.add)
                if not ln2_trivial:
                    nc.gpsimd.tensor_mul(res[:], res[:], g2row[:])
                    nc.gpsimd.tensor_add(res[:], res[:], b2row[:])
                # rows stay in sorted-token order; host unpermutes
                nc.sync.dma_start(out_d[ts(nt, P), :], res[:])

            # software-pipelined with skew 2: PE runs sampled-matmuls of
            # nt+1/nt+2 while the LN chain of nt completes
            SKEW = 3
            nF = NT if dbg_do_f else 0
            pend = []
            for nt in range(nF):
                pend.append((nt, stage_a(nt)))
                if len(pend) > SKEW:
                    j, e = pend.pop(0)
                    stage_b(j, e)
            for j, e in pend:
                stage_b(j, e)

    nc.compile()
    return nc


def host_prep(embeddings, positions, grid_points, pos_table, sigma, alpha,
              beta, dt, ln1_g, ln1_b, ln2_g, ln2_b, w_out, b_out):
    """Host-side prep: derived index tensors + per-core input maps."""
    embeddings = np.asarray(embeddings, np.float32)
    positions = np.asarray(positions, np.float32)
    grid_points = np.asarray(grid_points, np.float32)
    pos_table = np.ascontiguousarray(np.asarray(pos_table, np.float32))
    alpha = np.asarray(alpha, np.float32)
    # residual fold: out+enh = enh @ (W + I)
    w_out = np.ascontiguousarray(np.asarray(w_out, np.float32)
                                 + np.eye(D, dtype=np.float32))
    b_out = np.asarray(b_out, np.float32)
    sigma = np.float32(np.asarray(sigma))
    beta = np.float32(np.asarray(beta))
    dt = np.float32(np.asarray(dt))
    ln1_g = np.asarray(ln1_g, np.float32)
    ln1_b = np.asarray(ln1_b, np.float32)
    ln2_g = np.asarray(ln2_g, np.float32)
    ln2_b = np.asarray(ln2_b, np.float32)

    c_exp = float(-(np.float32(1.0) / (np.float32(2.0) * sigma * sigma)))
    scale = float(np.float32(1.0) / np.sqrt(np.float32(D)))
    beta_dt = float(beta * dt)
    alphadt = np.ascontiguousarray((dt * alpha).astype(np.float32)
                                   .reshape(D, 1))

    # Chebyshev factorization of the RBF kernel: K(u,p) = L(u) Kc L(p)^T,
    # exact to ~1e-15 at RK nodes (kernel is entire, sigma=0.2 wide)
    kq = np.arange(RK)
    tn = 0.5 + 0.5 * np.cos((2 * kq + 1) * np.pi / (2 * RK))
    bw = np.empty(RK)
    for j in range(RK):
        bw[j] = 1.0 / np.prod(tn[j] - np.delete(tn, j))

    def lagrange(x):
        diff = x[:, None] - tn[None, :]
        hit = np.isclose(diff, 0.0, atol=1e-14)
        diff = np.where(hit, 1.0, diff)
        num = bw[None, :] / diff
        L = num / num.sum(1, keepdims=True)
        rows = hit.any(1)
        L[rows] = hit[rows].astype(np.float64)
        return L

    ln1_trivial = bool(np.all(ln1_g == 1.0) and np.all(ln1_b == 0.0))
    ln2_trivial = bool(np.all(ln2_g == 1.0) and np.all(ln2_b == 0.0))
    bout_trivial = bool(np.all(b_out == 0.0))

    import ml_dtypes

    def to_f32r(x):
        xb = np.asarray(x, np.float32).view(np.uint32)
        sh = np.uint32(13)
        r = ((xb >> sh) + ((xb >> np.uint32(12)) & np.uint32(1))) << sh
        return r.view(np.float32)

    pt_bf16 = np.ascontiguousarray(pos_table.astype(ml_dtypes.bfloat16))
    giota = np.arange(MAXLEN, dtype=np.float32).reshape(MAXLEN, 1)
    in_maps = []
    all_bands = []
    all_pt_bands = []
    all_orders = []
    for c in range(NCORES):
        pos_n = positions[c, :, 0]                     # [N] fp32 (natural)
        u_n = pos_n * np.float32(G - 1)
        order = np.argsort(u_n, kind="stable").astype(np.int32)
        all_orders.append(order)
        pos = pos_n[order]                             # sorted token order
        u = u_n[order]
        idx = np.clip(np.rint(pos * np.float32(MAXLEN - 1)).astype(np.int32),
                      0, MAXLEN - 1)
        # grid blocks each sorted token tile touches (i0..i0+1 support)
        i0 = np.clip(np.floor(u).astype(np.int64), 0, G - 1)
        ihi = np.minimum(i0 + 1, G - 1)
        bands = []
        for nt in range(NT):
            lo = int(i0[nt * P:(nt + 1) * P].min()) // P
            hi = int(ihi[nt * P:(nt + 1) * P].max()) // P
            bands.append(tuple(range(lo, hi + 1)))
        all_bands.append(tuple(bands))
        ptb = []
        for nt in range(NT):
            lo = int(idx[nt * P:(nt + 1) * P].min()) // P
            hi = int(idx[nt * P:(nt + 1) * P].max()) // P
            ptb.append(tuple(range(lo, hi + 1)))
        all_pt_bands.append(tuple(ptb))
        u_g = grid_points[c, :, 0].astype(np.float64)
        amatT = (np.exp(-(tn[:, None] - tn[None, :]) ** 2
                        / (2.0 * float(sigma) ** 2)) @ lagrange(u_g).T)
        bmat = lagrange(pos.astype(np.float64))
        m = {
            "emb0": np.ascontiguousarray(embeddings[c][order]),
            "pt": pt_bf16,
            "bmat": np.ascontiguousarray(to_f32r(
                bmat.astype(np.float32).reshape(NT, P, RK)
                .transpose(1, 0, 2).reshape(P, NT * RK))),
            "amatT": np.ascontiguousarray(to_f32r(amatT)),
            "idxrow": np.ascontiguousarray(
                idx.astype(np.float32).reshape(1, N)),
            "urow": np.ascontiguousarray(u.reshape(1, N)),
            "giota": giota,
            "alphadt": alphadt,
            "wout": to_f32r(w_out),
        }
        if not ln1_trivial:
            m["ln1g"] = np.ascontiguousarray(ln1_g.reshape(1, D))
            m["ln1b"] = np.ascontiguousarray(ln1_b.reshape(1, D))
        if not ln2_trivial:
            m["ln2g"] = np.ascontiguousarray(ln2_g.reshape(1, D))
            m["ln2b"] = np.ascontiguousarray(ln2_b.reshape(1, D))
        if not bout_trivial:
            m["bout"] = np.ascontiguousarray(b_out.reshape(1, D))
        in_maps.append(m)

    # SPMD: one program for all cores -> per-tile band = union over cores
    bands = tuple(
        tuple(range(min(b[nt][0] for b in all_bands),
                    max(b[nt][-1] for b in all_bands) + 1))
        for nt in range(NT))
    pt_bands = tuple(
        tuple(range(min(b[nt][0] for b in all_pt_bands),
                    max(b[nt][-1] for b in all_pt_bands) + 1))
        for nt in range(NT))
    build_key = (c_exp, scale, beta_dt, ln1_trivial, ln2_trivial,
                 bout_trivial, bands, pt_bands)
    return in_maps, build_key, all_orders


_NC_CACHE = {}


def kernel(**inputs):
    in_maps, build_key, orders = host_prep(**inputs)
    if build_key not in _NC_CACHE:
        _NC_CACHE[build_key] = build_nc(*build_key)
    nc = _NC_CACHE[build_key]
    res = run_bass_kernel_spmd(nc, in_maps, list(range(NCORES)))
    out = np.empty((NCORES, N, D), np.float32)
    for i in range(NCORES):
        out[i, orders[i], :] = res.results[i]["out"]
    return out

